# revision 1
# baseline (speedup 1.0000x reference)
"""Trainium2 Bass kernel for nn_Attention_49134425866421.

Dense transformer attention block:
  qkv = x @ W_qkv + b_qkv  -> partial RoPE on q,k -> softmax attention -> out proj.

Shapes (hardcoded): B=4, N=2048, C=768, H=12, D=64, fp32.

Sharding: 8 cores = (batch b in 0..3) x (head-group g in 0..1, 6 heads each).
Each core computes q/k/v projections for its 6 heads, attention, and a partial
output projection (row-parallel over head dims). Host sums the two partials
per batch and adds b_proj.

On-chip layouts (per core):
  xT    [128,6,2048]  x[b]^T, contraction dim c on partitions (c = ko*128+p)
  qT,kT [128,3,2048]  per-head-pair: partition p = 64*(h%2)+d, free (hp, t)
  V     [128,16,6,65] natural: partition = t%128, free (t//128, local head, d)
                      column 64 holds ones -> AV matmul also produces rowsums
  attnT [128,3,2048]  bf16, same layout as qT -> feeds row-parallel proj

RoPE trick: rotate_half is a cross-partition half-swap; done via SBUF->SBUF
DMA of (q * m2s) where m2s = pre-swapped signed sin table, so
q_rope = q*cos + swap(q*m2s). Special (non-rotated) tokens handled by padding
cos=1,sin=0 rows host-side. Softmax without max-subtraction (scores are
N(0,~1); exp never overflows); scale 1/8 folded into the ACT exp call;
rowsum via the ones-column of V'.
"""

import os
import sys

import numpy as np

try:
    import concourse.bass as bass  # noqa: F401
except ImportError:
    sys.path.insert(0, "/opt/trn_rl_repo")

import ml_dtypes

B, N, C, H, D = 4, 2048, 768, 12, 64
HPC = 6          # heads per core
NPAIR = 3        # head pairs per core
P = 128
NT = N // P      # 16 token tiles
TC = 512         # token chunk for matmul free dim
NTC = N // TC    # 4

_NC_CACHE = {}
LAST_RESULTS = None  # BassKernelResults stash for test.py


def _build_nc():
    from contextlib import ExitStack

    import concourse.bass as bass
    import concourse.bacc as bacc
    import concourse.mybir as mybir
    import concourse.tile as tile

    f32 = mybir.dt.float32
    f32r = mybir.dt.float32r
    bf16 = mybir.dt.bfloat16
    EXP = mybir.ActivationFunctionType.Exp

    nc = bacc.Bacc(None, target_bir_lowering=False)

    xT_d = nc.dram_tensor("xT", [C, N], f32r, kind="ExternalInput")
    wqk_d = nc.dram_tensor("w_qk", [P, 6, 768], f32r, kind="ExternalInput")
    wv_d = nc.dram_tensor("w_v", [P, 6, 384], f32r, kind="ExternalInput")
    wp_d = nc.dram_tensor("w_p", [P, 3, 768], bf16, kind="ExternalInput")
    bqk_d = nc.dram_tensor("b_qk", [1, 768], f32r, kind="ExternalInput")
    bv_d = nc.dram_tensor("b_v", [1, 384], f32r, kind="ExternalInput")
    ones_d = nc.dram_tensor("ones", [1, TC], f32r, kind="ExternalInput")
    bqkt_d = nc.dram_tensor("b_qk_t", [P, 6], f32, kind="ExternalInput")
    cos_d = nc.dram_tensor("cos_tab", [P, N], f32, kind="ExternalInput")
    m2s_d = nc.dram_tensor("m2s_tab", [P, N], f32, kind="ExternalInput")
    y_d = nc.dram_tensor("y", [N, C], f32, kind="ExternalOutput")

    with tile.TileContext(nc) as tc, ExitStack() as ctx:
        singles = ctx.enter_context(tc.tile_pool(name="singles", bufs=1))
        mm_ps = ctx.enter_context(tc.tile_pool(name="mm_ps", bufs=2, space="PSUM"))
        att_ps = ctx.enter_context(tc.tile_pool(name="att_ps", bufs=2, space="PSUM"))
        acc_ps = ctx.enter_context(tc.tile_pool(name="acc_ps", bufs=1, space="PSUM"))
        rope_tmp = ctx.enter_context(tc.tile_pool(name="rope_tmp", bufs=2))
        pt_pool = ctx.enter_context(tc.tile_pool(name="pt", bufs=3))
        rb_pool = ctx.enter_context(tc.tile_pool(name="rb", bufs=2))
        y_pool = ctx.enter_context(tc.tile_pool(name="yout", bufs=2))

        # ---- static SBUF tensors ----
        xT = singles.tile([P, 6, N], f32r)
        wqk = singles.tile([P, 6, 768], f32r)
        wv = singles.tile([P, 6, 384], f32r)
        wp = singles.tile([P, 3, 768], bf16)
        bqk = singles.tile([1, 768], f32r)
        bv = singles.tile([1, 384], f32r)
        cosT = singles.tile([P, N], f32)
        m2sT = singles.tile([P, N], f32)
        ones = singles.tile([1, TC], f32r)
        bqkt = singles.tile([P, 6], f32)
        qT = singles.tile([P, NPAIR, N], f32r)
        kT = singles.tile([P, NPAIR, N], f32r)
        Vt = singles.tile([P, NT, HPC, D + 1], bf16)
        attnT = singles.tile([P, NPAIR, N], bf16)

        xT_r = xT_d.rearrange("(ko p) t -> p ko t", p=P)
        for ko in range(6):
            nc.sync.dma_start(xT[:, ko, :], xT_r[:, ko, :])
        nc.scalar.dma_start(wqk[:], wqk_d[:])
        nc.sync.dma_start(wv[:], wv_d[:])
        nc.scalar.dma_start(bqk[:], bqk_d[:])
        nc.sync.dma_start(bv[:], bv_d[:])
        nc.sync.dma_start(ones[:], ones_d[:])
        nc.sync.dma_start(bqkt[:], bqkt_d[:])
        nc.scalar.dma_start(cosT[:], cos_d[:])
        nc.scalar.dma_start(m2sT[:], m2s_d[:])
        nc.gpsimd.memset(Vt[:], 1.0)

        def emit_qk(hp):
            for tcu in range(NTC):
                tsl = slice(tcu * TC, (tcu + 1) * TC)
                for mt in (3 + hp, hp):  # k pair first, then q pair
                    dst = qT if mt < 3 else kT
                    ps = mm_ps.tile([P, TC], f32, tag="mm")
                    for ko in range(6):
                        nc.tensor.matmul(
                            ps,
                            lhsT=wqk[:, ko, mt * P : (mt + 1) * P],
                            rhs=xT[:, ko, tsl],
                            start=(ko == 0),
                            stop=(ko == 5),
                        )
                    # bias add on DVE, then rope: dst = pb*cos + swap(pb*m2s)
                    pb = rope_tmp.tile([P, TC], f32, tag="pb")
                    qs = rope_tmp.tile([P, TC], f32, tag="qs")
                    qsw = rope_tmp.tile([P, TC], f32, tag="qsw")
                    nc.vector.tensor_scalar_add(
                        out=pb[:], in0=ps[:], scalar1=bqkt[:, mt : mt + 1]
                    )
                    nc.vector.tensor_mul(out=qs[:], in0=pb[:], in1=m2sT[:, tsl])
                    nc.vector.tensor_mul(
                        out=dst[:, hp, tsl], in0=pb[:], in1=cosT[:, tsl]
                    )
                    for blk in range(4):
                        sp = [1, 0, 3, 2][blk] * 32
                        nc.sync.dma_start(
                            out=qsw[blk * 32 : blk * 32 + 32, :],
                            in_=qs[sp : sp + 32, :],
                        )
                    nc.vector.tensor_add(
                        out=dst[:, hp, tsl], in0=dst[:, hp, tsl], in1=qsw[:]
                    )

        emit_qk(0)
        nc.sync.dma_start(wp[:], wp_d[:])

        # ---- V projection (natural layout), all 6 heads; emitted per
        # token-tile, interleaved into the first attention pass ----
        def emit_v(tt):
            ps = mm_ps.tile([P, TC], f32, tag="mm")
            vps = ps[:, :384]
            for ko in range(6):
                nc.tensor.matmul(
                    vps,
                    lhsT=xT[:, ko, tt * P : (tt + 1) * P],
                    rhs=wv[:, ko, :],
                    start=(ko == 0),
                    stop=False,
                )
            nc.tensor.matmul(
                vps, lhsT=ones[:, :P], rhs=bv[:], start=False, stop=True
            )
            nc.vector.tensor_copy(
                out=Vt[:, tt, :, :D],
                in_=vps.rearrange("p (h d) -> p h d", h=HPC),
            )

        # ---- per head-pair: q/k projection + RoPE, then attention ----
        for hp in range(NPAIR):
            if hp > 0:
                emit_qk(hp)

            # attention for the two heads of this pair
            for ic in range(NTC):
                isl = slice(ic * TC, (ic + 1) * TC)
                accA = acc_ps.tile([D + 1, TC], f32, tag="accA")
                accB = acc_ps.tile([D + 1, TC], f32, tag="accB")
                for jt in range(NT):
                    if hp == 0 and ic == 0:
                        emit_v(jt)
                    st = att_ps.tile([P, 2 * TC], f32, tag="st")
                    nc.tensor.matmul(
                        st[:, :TC],
                        lhsT=kT[:D, hp, jt * P : (jt + 1) * P],
                        rhs=qT[:D, hp, isl],
                        start=True,
                        stop=True,
                        tile_position=(0, 0),
                    )
                    nc.tensor.matmul(
                        st[:, TC:],
                        lhsT=kT[D:, hp, jt * P : (jt + 1) * P],
                        rhs=qT[D:, hp, isl],
                        start=True,
                        stop=True,
                        tile_position=(64, 0),
                    )
                    pt = pt_pool.tile([P, 2 * TC], bf16, tag="pt")
                    if os.environ.get("ABLATE") == "exp":
                        nc.scalar.activation(pt[:, :8], st[:, :8], EXP, scale=0.125)
                        nc.scalar.activation(pt[:, 8:], st[:, 8:], EXP, scale=0.125) if False else None
                    else:
                        nc.scalar.activation(pt[:], st[:], EXP, scale=0.125)
                    nc.tensor.matmul(
                        accA,
                        lhsT=Vt[:, jt, 2 * hp, :],
                        rhs=pt[:, :TC],
                        start=(jt == 0),
                        stop=(jt == NT - 1),
                    )
                    nc.tensor.matmul(
                        accB,
                        lhsT=Vt[:, jt, 2 * hp + 1, :],
                        rhs=pt[:, TC:],
                        start=(jt == 0),
                        stop=(jt == NT - 1),
                    )
                # evacuate PSUM accs to SBUF immediately so the banks free up
                # for the next i-chunk; rescale then runs off the PE critical
                # path entirely.
                accs = []
                for half, acc in ((0, accA), (1, accB)):
                    asb = rb_pool.tile([D + 1, TC], f32, tag="asb")
                    nc.vector.tensor_copy(out=asb[:], in_=acc[:])
                    accs.append(asb)
                for half, asb in ((0, accs[0]), (1, accs[1])):
                    rec = rb_pool.tile([1, TC], f32, tag="rec")
                    rbc = rb_pool.tile([D, TC], f32, tag="rbc")
                    nc.vector.reciprocal(out=rec[:], in_=asb[D : D + 1, :])
                    nc.gpsimd.partition_broadcast(rbc[:], rec[:], channels=D)
                    nc.vector.tensor_mul(
                        out=attnT[half * D : (half + 1) * D, hp, isl],
                        in0=asb[:D, :],
                        in1=rbc[:],
                    )

        # ---- output projection (row-parallel partial) ----
        for tt in range(NT):
            for ch in range(2):
                ps = mm_ps.tile([P, TC], f32, tag="mm")
                yps = ps[:, :384]
                for ko in range(3):
                    nc.tensor.matmul(
                        yps,
                        lhsT=attnT[:, ko, tt * P : (tt + 1) * P],
                        rhs=wp[:, ko, ch * 384 : (ch + 1) * 384],
                        start=(ko == 0),
                        stop=(ko == 2),
                    )
                yt = y_pool.tile([P, 384], f32, tag="yt")
                nc.vector.tensor_copy(out=yt[:], in_=yps)
                nc.sync.dma_start(
                    out=y_d[tt * P : (tt + 1) * P, ch * 384 : (ch + 1) * 384],
                    in_=yt[:],
                )

    nc.finalize()
    return nc


def _host_inputs(x, rope_cos, rope_sin, W_qkv, b_qkv, W_proj, b_proj, num_special):
    ns = int(num_special)
    cos_pad = np.ones((N, D), np.float32)
    sin_pad = np.zeros((N, D), np.float32)
    cos_pad[ns:] = rope_cos
    sin_pad[ns:] = rope_sin
    # m2s[t, d] = +sin[t, d+32] (d<32) else -sin[t, d-32]
    m2s = np.empty_like(sin_pad)
    m2s[:, : D // 2] = sin_pad[:, D // 2 :]
    m2s[:, D // 2 :] = -sin_pad[:, : D // 2]
    cos_tab = np.tile(np.ascontiguousarray(cos_pad.T), (2, 1))
    m2s_tab = np.tile(np.ascontiguousarray(m2s.T), (2, 1))

    in_maps = []
    for core in range(8):
        b, g = core // 2, core % 2
        hs = list(range(HPC * g, HPC * g + HPC))
        cols_qk = []
        for mt in range(6):
            s, hp = (0, mt) if mt < 3 else (1, mt - 3)
            for half in range(2):
                h = hs[2 * hp + half]
                cols_qk.extend(s * 768 + h * 64 + d for d in range(D))
        cols_qk = np.array(cols_qk)
        cols_v = np.array([2 * 768 + hs[i // 64] * 64 + (i % 64) for i in range(384)])
        rows_p = np.array(
            [hs[2 * ko + half] * 64 + d
             for ko in range(3) for half in range(2) for d in range(D)]
        )
        in_maps.append({
            "xT": np.ascontiguousarray(x[b].T),
            "w_qk": np.ascontiguousarray(
                W_qkv[:, cols_qk].reshape(6, P, 768).transpose(1, 0, 2)),
            "w_v": np.ascontiguousarray(
                W_qkv[:, cols_v].reshape(6, P, 384).transpose(1, 0, 2)),
            "w_p": np.ascontiguousarray(
                W_proj[rows_p].reshape(3, P, 768).transpose(1, 0, 2)
            ).astype(ml_dtypes.bfloat16),
            "b_qk": np.ascontiguousarray(b_qkv[cols_qk].reshape(1, 768)),
            "b_qk_t": np.ascontiguousarray(
                b_qkv[cols_qk].reshape(6, P).T),
            "b_v": np.ascontiguousarray(b_qkv[cols_v].reshape(1, 384)),
            "ones": np.ones((1, TC), np.float32),
            "cos_tab": cos_tab,
            "m2s_tab": m2s_tab,
        })
    return in_maps


def kernel(x, rope_cos, rope_sin, W_qkv, b_qkv, W_proj, b_proj, num_special):
    global LAST_RESULTS
    from concourse.bass_utils import run_bass_kernel_spmd

    x = np.asarray(x, np.float32)
    if "nc" not in _NC_CACHE:
        _NC_CACHE["nc"] = _build_nc()
    nc = _NC_CACHE["nc"]

    in_maps = _host_inputs(
        x, np.asarray(rope_cos, np.float32), np.asarray(rope_sin, np.float32),
        np.asarray(W_qkv, np.float32), np.asarray(b_qkv, np.float32),
        np.asarray(W_proj, np.float32), np.asarray(b_proj, np.float32), num_special,
    )
    trace = bool(int(os.environ.get("KERNEL_TRACE", "0")))
    res = run_bass_kernel_spmd(nc, in_maps, core_ids=list(range(8)), trace=trace)
    LAST_RESULTS = res

    bp = np.asarray(b_proj, np.float32)
    out = np.empty((B, N, C), np.float32)
    for b in range(B):
        out[b] = res.results[2 * b]["y"] + res.results[2 * b + 1]["y"] + bp
    return out



# revision 35
# speedup vs baseline: 1.1745x; 1.1745x over previous
"""Trainium2 Bass kernel for nn_Attention_49134425866421 (v3).

Dense transformer attention block:
  qkv = x @ W_qkv + b_qkv -> partial RoPE on q,k -> softmax attention -> out proj.

Shapes (hardcoded): B=4, N=2048, C=768, H=12, D=64, fp32 in/out.

Sharding: 8 cores = (batch b in 0..3) x (head-group g in 0..1, 6 heads each).
Host sums the two partials per batch and adds b_proj + b_v @ W_proj (softmax
rows sum to 1, so the V bias contributes exactly b_v @ W_proj - host side).

Design (vs 304us v1 baseline):
 - all-bf16 datapath; qk bias folded into the projection matmul as a 7th
   accumulation step (ones-row x bias-row), so PSUM evacuation is a plain
   ACT/DVE copy and rope is 2 DVE muls + 1 DVE stream_shuffle + 1 Pool add.
 - RoPE rotate_half via stream_shuffle: head-dim layout permuted host-side
   (P64) so rotate pairs sit 16 partitions apart within one 32-block.
 - attention in two global q-half passes (q 0:1024, 1024:2048). Per (pass, h,
   jt): 2 score matmuls -> [128,1024] PSUM (2 banks, double-buffered) -> one
   1024-wide exp -> 8 transposed-AV matmuls (free dim 65, accumulating
   [q128, d+rowsum] over jt; rowsum rides V's ones column).
 - per-(h,pass) chain: reciprocal + per-subtile normalize (per-partition
   scalar), PE transpose back to head-major attnT. Chain pieces and deferred
   work (V tiles 4..15, qk pairs 1,2) are drip-fed one piece per attention
   step into PE/DVE slack.
 - first-half projection runs during pass 2; only the second half's
   projection is tail-exposed. y is bf16, summed on host.
PSUM: st 2x2 banks + acc 2 + trp 1 + pj 1 = 8.
"""

import os
import sys

import numpy as np

try:
    import concourse.bass as bass  # noqa: F401
except ImportError:
    sys.path.insert(0, "/opt/trn_rl_repo")

import ml_dtypes

B, N, C, H, D = 4, 2048, 768, 12, 64
HPC = 6          # heads per core
NPAIR = 3
P = 128
NT = N // P      # 16 token tiles
TC = 512
NTC = N // TC    # 4
QH = 1024        # q-half width
NSUB = 8         # q-subtiles per pass

P64 = np.concatenate([np.arange(0, 16), np.arange(32, 48),
                      np.arange(16, 32), np.arange(48, 64)])
SHUF_MASK = [(i + 16) % 32 for i in range(32)]

_NC_CACHE = {}
LAST_RESULTS = None


def _build_nc():
    from contextlib import ExitStack

    import concourse.bass as bass
    import concourse.bacc as bacc
    import concourse.mybir as mybir
    import concourse.tile as tile

    f32 = mybir.dt.float32
    bf16 = mybir.dt.bfloat16
    EXP = mybir.ActivationFunctionType.Exp
    IDENT = mybir.ActivationFunctionType.Identity

    nc = bacc.Bacc(None, target_bir_lowering=False)

    xT_d = nc.dram_tensor("xT", [C, N], bf16, kind="ExternalInput")
    wqk_d = nc.dram_tensor("w_qk", [P, 6, 768], bf16, kind="ExternalInput")
    wv_d = nc.dram_tensor("w_v", [P, 6, 384], bf16, kind="ExternalInput")
    wp_d = nc.dram_tensor("w_p", [P, 3, 768], bf16, kind="ExternalInput")
    bqkt_d = nc.dram_tensor("b_qk_t", [P, 6], f32, kind="ExternalInput")
    cos_d = nc.dram_tensor("cos_tab", [P, N], bf16, kind="ExternalInput")
    m2s_d = nc.dram_tensor("m2s_tab", [P, N], bf16, kind="ExternalInput")
    ident_d = nc.dram_tensor("ident", [P, P], bf16, kind="ExternalInput")
    y_d = nc.dram_tensor("y", [N, C], bf16, kind="ExternalOutput")
    debug = bool(int(os.environ.get("KERNEL_DEBUG", "0")))
    if debug:
        dbg_q = nc.dram_tensor("dbg_q", [P, NPAIR, N], bf16, kind="ExternalOutput")
        dbg_k = nc.dram_tensor("dbg_k", [P, NPAIR, N], bf16, kind="ExternalOutput")
        dbg_v = nc.dram_tensor("dbg_v", [P, NT, HPC, D + 1], bf16,
                               kind="ExternalOutput")
        dbg_a = nc.dram_tensor("dbg_a", [P, NPAIR, N], bf16, kind="ExternalOutput")

    xT_r = xT_d.rearrange("(ko p) t -> p ko t", p=P)

    with tile.TileContext(nc) as tc, ExitStack() as ctx:
        singles = ctx.enter_context(tc.tile_pool(name="singles", bufs=1))
        rope_p = ctx.enter_context(tc.tile_pool(name="rope", bufs=2))
        pt_p = ctx.enter_context(tc.tile_pool(name="ptp", bufs=36))
        an_p = ctx.enter_context(tc.tile_pool(name="anp", bufs=10))
        yt_p = ctx.enter_context(tc.tile_pool(name="ytp", bufs=4))
        rec_p = ctx.enter_context(tc.tile_pool(name="recp", bufs=2))
        st_p = ctx.enter_context(tc.tile_pool(name="stp", bufs=2, space="PSUM"))
        acc_p = ctx.enter_context(tc.tile_pool(name="accp", bufs=1, space="PSUM"))
        aux_p = ctx.enter_context(tc.tile_pool(name="auxp", bufs=1, space="PSUM"))
        pj_p = ctx.enter_context(tc.tile_pool(name="pjp", bufs=1, space="PSUM"))

        xT = singles.tile([P, 6, N], bf16)
        wqk = singles.tile([P, 6, 768], bf16)
        wv = singles.tile([P, 6, 384], bf16)
        wp = singles.tile([P, 3, 768], bf16)
        bqkt = singles.tile([P, 6], f32)
        cosT = singles.tile([P, N], bf16)
        m2sT = singles.tile([P, N], bf16)
        ident = singles.tile([P, P], bf16)
        qT = singles.tile([P, NPAIR, N], bf16)
        kT = singles.tile([P, NPAIR, N], bf16)
        Vt = singles.tile([P, NT, HPC, D + 1], bf16)
        attnT = singles.tile([P, NPAIR, N], bf16)

        # ---- input DMAs: wqk on ACT queue; xT+tables chunk-interleaved on SP
        # so device-FIFO arrival matches phase-A consumption order ----
        nc.scalar.dma_start(bqkt[:], bqkt_d[:])
        for ko in range(6):
            nc.scalar.dma_start(wqk[:, ko, :], wqk_d[:, ko, :])
        for tcu in range(NTC):
            tsl = slice(tcu * TC, (tcu + 1) * TC)
            for ko in range(6):
                nc.sync.dma_start(xT[:, ko, tsl], xT_r[:, ko, tsl])
            nc.sync.dma_start(cosT[:, tsl], cos_d[:, tsl])
            nc.sync.dma_start(m2sT[:, tsl], m2s_d[:, tsl])
        nc.sync.dma_start(wv[:], wv_d[:])
        nc.sync.dma_start(ident[:], ident_d[:])
        nc.sync.dma_start(wp[:], wp_d[:])

        nc.gpsimd.memset(Vt[:], 1.0)

        def qk_mms(ps, mt, tsl):
            for ko in range(6):
                nc.tensor.matmul(
                    ps, lhsT=wqk[:, ko, mt * P : (mt + 1) * P],
                    rhs=xT[:, ko, tsl], start=(ko == 0), stop=(ko == 5))

        def rope_chain(ps, mt, tsl, dst, pb_engine):
            # PSUM evacuation + per-partition qk bias in one op
            pb = rope_p.tile([P, TC], bf16, tag="pb", bufs=12)
            if pb_engine == "act":
                nc.scalar.activation(pb[:], ps[:], IDENT,
                                     bias=bqkt[:, mt : mt + 1])
            else:
                nc.vector.tensor_scalar_add(out=pb[:], in0=ps[:],
                                            scalar1=bqkt[:, mt : mt + 1])
            qs = rope_p.tile([P, TC], bf16, tag="qs")
            qsw = rope_p.tile([P, TC], bf16, tag="qsw")
            nc.vector.tensor_mul(out=qs[:], in0=pb[:], in1=m2sT[:, tsl])
            nc.vector.tensor_mul(out=dst[:], in0=pb[:], in1=cosT[:, tsl])
            nc.vector.stream_shuffle(qsw[:], qs[:], SHUF_MASK)
            nc.gpsimd.tensor_add(out=dst[:], in0=dst[:], in1=qsw[:])

        def v_mms(ps, jt, ko_range):
            for ko in ko_range:
                nc.tensor.matmul(
                    ps[:, :384], lhsT=xT[:, ko, jt * P : (jt + 1) * P],
                    rhs=wv[:, ko, :], start=(ko == 0), stop=(ko == 5))

        def v_evac(ps, jt, engine):
            out = Vt[:, jt, :, : D]
            src = ps[:, :384].rearrange("p (h d) -> p h d", h=HPC)
            if engine == "act":
                nc.scalar.copy(out=out, in_=src)
            else:
                nc.vector.tensor_copy(out=out, in_=src)

        # ======== phase A: qk pairs 0,1 + V[12..15] ========
        for tcu in range(NTC):
            tsl = slice(tcu * TC, (tcu + 1) * TC)
            for mt in (3, 0, 4, 1):  # pair0 k, q; pair1 k, q
                dst = qT if mt < 3 else kT
                hp = mt if mt < 3 else mt - 3
                ps = st_p.tile([P, QH], f32, tag="st", name="psA")
                qk_mms(ps[:, :TC], mt, tsl)
                rope_chain(ps[:, :TC], mt, tsl, dst[:, hp, tsl], "act")
            if tcu == 3:
                for jt in range(12, NT):
                    ps = st_p.tile([P, QH], f32, tag="st", name="psV")
                    v_mms(ps, jt, range(6))
                    v_evac(ps, jt, "act")

        # =================== deferred-work pieces ===================
        urgent = []
        lazy = []

        dp_ctr = [0]

        def defer_ps():
            # alternate deferred-piece PSUM between the pj and aux banks so
            # two pieces are in flight and the PE never waits on a bank free
            # (aux shares its slot with the chain trp tiles via the same tag)
            dp_ctr[0] += 1
            if dp_ctr[0] % 2:
                return pj_p.tile([P, TC], f32, tag="pj", name="dps")
            return aux_p.tile([P, TC], f32, tag="trp", name="dps")

        def defer_v(jt):
            state = {}
            def part1():
                ps = defer_ps()
                state["ps"] = ps
                for ko in range(3):
                    nc.tensor.matmul(
                        ps[:, :384], lhsT=xT[:, ko, jt * P : (jt + 1) * P],
                        rhs=wv[:, ko, :], start=(ko == 0), stop=False)
            def part2():
                ps = state["ps"]
                for ko in range(3, 6):
                    nc.tensor.matmul(
                        ps[:, :384], lhsT=xT[:, ko, jt * P : (jt + 1) * P],
                        rhs=wv[:, ko, :], start=False, stop=(ko == 5))
                v_evac(ps, jt, "dve")
            urgent.append(part1)
            urgent.append(part2)

        def defer_qk(mt, tcu):
            tsl = slice(tcu * TC, (tcu + 1) * TC)
            dst = qT if mt < 3 else kT
            hp = mt if mt < 3 else mt - 3
            def piece():
                ps = defer_ps()
                qk_mms(ps, mt, tsl)
                rope_chain(ps, mt, tsl, dst[:, hp, tsl], "dve")
            lazy.append(piece)

        for jt in range(12):
            defer_v(jt)
        for mt in (5, 2):       # pair 2 k, q
            for tcu in range(NTC):
                defer_qk(mt, tcu)

        def proj_piece(tt, ch, pool, stage_engine):
            def piece():
                if pool is pj_p:
                    ps = pj_p.tile([P, TC], f32, tag="pj", name="pjps")
                else:
                    ps = st_p.tile([P, QH], f32, tag="st", name="pjst")
                for ko in range(3):
                    nc.tensor.matmul(
                        ps[:, :384],
                        lhsT=attnT[:, ko, tt * P : (tt + 1) * P],
                        rhs=wp[:, ko, ch * 384 : (ch + 1) * 384],
                        start=(ko == 0), stop=(ko == 2))
                yt = yt_p.tile([P, 384], bf16, tag="yt")
                if stage_engine == "act":
                    nc.scalar.copy(out=yt[:], in_=ps[:, :384])
                else:
                    nc.vector.tensor_copy(out=yt[:], in_=ps[:, :384])
                nc.sync.dma_start(
                    out=y_d[tt * P : (tt + 1) * P, ch * 384 : (ch + 1) * 384],
                    in_=yt[:])
            return piece

        # ============== attention: two q-half passes ==============
        # PSUM accumulation groups zero a whole 2KB bank on start, so the 8
        # AV accumulations per (head, pass) run SEQUENTIALLY (subtile-outer,
        # jt-inner) over the two acc banks. The jt loop emits scores+exp only
        # (keeping all 16 pt tiles); AV groups + normalize/transpose chains
        # are drip-fed into the next head's steps.
        def av_group(h, i, accs, pts, three_way=False):
            def piece():
                if three_way and i % 3 == 2:
                    acc = pj_p.tile([P, D + 1], f32, tag="pj", name="acc")
                elif three_way:
                    acc = acc_p.tile([P, D + 1], f32, tag=f"a{i % 3}", name="acc")
                else:
                    acc = acc_p.tile([P, D + 1], f32, tag=f"a{i % 2}", name="acc")
                accs[i] = acc
                for jt in range(NT):
                    nc.tensor.matmul(
                        acc[:], lhsT=pts[jt][:, i * P : (i + 1) * P],
                        rhs=Vt[:, jt, h, :],
                        start=(jt == 0), stop=(jt == NT - 1))
            return piece

        def chain_piece(h, psx, i, accs):
            hp, base = h // 2, 64 * (h % 2)
            tt = psx * NSUB + i
            def piece():
                acc = accs[i]
                rec = rec_p.tile([P, 1], f32, tag="rec", name="rec")
                nc.vector.reciprocal(out=rec[:], in_=acc[:, D : D + 1])
                anorm = an_p.tile([P, D], bf16, tag="an", name="anorm")
                nc.vector.tensor_scalar_mul(
                    out=anorm[:], in0=acc[:, :D], scalar1=rec[:])
                trp = aux_p.tile([P, P], bf16, tag="trp", name="trp")
                nc.tensor.transpose(
                    trp[base : base + D, :], anorm[:], ident[:],
                    tile_position=(0, base))
                nc.vector.tensor_copy(
                    out=attnT[base : base + D, hp, tt * P : (tt + 1) * P],
                    in_=trp[base : base + D, :])
            return piece

        for psx in range(2):
            for h in range(HPC):
                hp, base = h // 2, 64 * (h % 2)
                pts = []
                for jt in range(NT):
                    st = st_p.tile([P, QH], f32, tag="st", name="st")
                    for qc in range(2):
                        q0 = psx * QH + qc * TC
                        nc.tensor.matmul(
                            st[:, qc * TC : (qc + 1) * TC],
                            lhsT=kT[base : base + D, hp, jt * P : (jt + 1) * P],
                            rhs=qT[base : base + D, hp, q0 : q0 + TC],
                            start=True, stop=True,
                            tile_position=(base, 0))
                    pt = pt_p.tile([P, QH], bf16, tag="pt")
                    nc.scalar.activation(pt[:], st[:], EXP, scale=0.125)
                    pts.append(pt)
                    if urgent:
                        urgent.pop(0)()
                    step_idx = (psx * HPC + h) * NT + jt
                    if lazy and step_idx % 2 == 1:
                        lazy.pop(0)()
                accs = {}
                if psx == 1 and h == HPC - 1:
                    av_group(h, 0, accs, pts, three_way=True)()
                    av_group(h, 1, accs, pts, three_way=True)()
                    tail_work = (h, psx, accs, pts)
                else:
                    for i in range(NSUB):
                        urgent.append(av_group(h, i, accs, pts))
                        urgent.append(chain_piece(h, psx, i, accs))
                if psx == 1 and h == 1:
                    # first-half projection: attnT[0:8 tiles] complete
                    for tt in range(NSUB):
                        for ch in range(2):
                            lazy.append(proj_piece(tt, ch, pj_p, "dve"))

        # ---- tail: h5/pass2 AV groups 2..7 + chain + 2nd-half projection;
        # groups 0,1 already ran inline, so chain/proj for them start at once
        # while group i+2 accumulates in the bank freed by chain i ----
        h, psx, accs, pts = tail_work
        av_group(h, 2, accs, pts, three_way=True)()
        chain_piece(h, psx, 0, accs)()
        for i in range(NSUB):
            if i + 1 < NSUB:
                chain_piece(h, psx, i + 1, accs)()
            if i + 3 < NSUB:
                av_group(h, i + 3, accs, pts, three_way=True)()
            tt = NSUB + i
            pj = st_p.tile([P, QH], f32, tag="st", name="pjt")
            for ch in range(2):
                for ko in range(3):
                    nc.tensor.matmul(
                        pj[:, ch * TC : ch * TC + 384],
                        lhsT=attnT[:, ko, tt * P : (tt + 1) * P],
                        rhs=wp[:, ko, ch * 384 : (ch + 1) * 384],
                        start=(ko == 0), stop=(ko == 2))
            yt = yt_p.tile([P, 768], bf16, tag="yt2", name="yt2")
            src_ap = pj[:, : 2 * TC].rearrange("p (b x) -> p b x", b=2)[:, :, :384]
            eng = nc.scalar if i % 2 else nc.vector
            if i % 2:
                nc.scalar.copy(out=yt.rearrange("p (b x) -> p b x", b=2), in_=src_ap)
            else:
                nc.vector.tensor_copy(out=yt.rearrange("p (b x) -> p b x", b=2),
                                      in_=src_ap)
            nc.sync.dma_start(out=y_d[tt * P : (tt + 1) * P, :], in_=yt[:])
        while urgent:
            urgent.pop(0)()
        while lazy:
            lazy.pop(0)()

        if debug:
            nc.sync.dma_start(dbg_q[:], qT[:])
            nc.sync.dma_start(dbg_k[:], kT[:])
            nc.sync.dma_start(dbg_v[:], Vt[:])
            nc.sync.dma_start(dbg_a[:], attnT[:])

    nc.finalize()
    return nc


def _host_inputs(x, rope_cos, rope_sin, W_qkv, b_qkv, W_proj, b_proj, num_special):
    ns = int(num_special)
    bf = ml_dtypes.bfloat16
    cos_pad = np.ones((N, D), np.float32)
    sin_pad = np.zeros((N, D), np.float32)
    cos_pad[ns:] = rope_cos
    sin_pad[ns:] = rope_sin
    m2s = np.empty_like(sin_pad)
    m2s[:, : D // 2] = sin_pad[:, D // 2 :]
    m2s[:, D // 2 :] = -sin_pad[:, : D // 2]
    cos_tab = np.tile(np.ascontiguousarray(cos_pad.T[P64]), (2, 1)).astype(bf)
    m2s_tab = np.tile(np.ascontiguousarray(m2s.T[P64]), (2, 1)).astype(bf)
    ident = np.eye(P, dtype=np.float32).astype(bf)

    in_maps = []
    for core in range(8):
        b, g = core // 2, core % 2
        hs = list(range(HPC * g, HPC * g + HPC))
        cols_qk = []
        for mt in range(6):
            s, hp = (0, mt) if mt < 3 else (1, mt - 3)
            for half in range(2):
                h = hs[2 * hp + half]
                cols_qk.extend(s * 768 + h * 64 + int(P64[p]) for p in range(D))
        cols_qk = np.array(cols_qk)
        cols_v = np.array([2 * 768 + hs[i // 64] * 64 + (i % 64) for i in range(384)])
        rows_p = np.array(
            [hs[2 * ko + half] * 64 + d
             for ko in range(3) for half in range(2) for d in range(D)]
        )
        in_maps.append({
            "xT": np.ascontiguousarray(x[b].T).astype(bf),
            "w_qk": np.ascontiguousarray(
                W_qkv[:, cols_qk].reshape(6, P, 768).transpose(1, 0, 2)).astype(bf),
            "w_v": np.ascontiguousarray(
                W_qkv[:, cols_v].reshape(6, P, 384).transpose(1, 0, 2)).astype(bf),
            "w_p": np.ascontiguousarray(
                W_proj[rows_p].reshape(3, P, 768).transpose(1, 0, 2)).astype(bf),
            "b_qk_t": np.ascontiguousarray(b_qkv[cols_qk].reshape(6, P).T),
            "cos_tab": cos_tab,
            "m2s_tab": m2s_tab,
            "ident": ident,
        })
    return in_maps


def kernel(x, rope_cos, rope_sin, W_qkv, b_qkv, W_proj, b_proj, num_special):
    global LAST_RESULTS
    from concourse.bass_utils import run_bass_kernel_spmd

    x = np.asarray(x, np.float32)
    W_qkv = np.asarray(W_qkv, np.float32)
    b_qkv = np.asarray(b_qkv, np.float32)
    W_proj = np.asarray(W_proj, np.float32)
    b_proj = np.asarray(b_proj, np.float32)
    if "nc" not in _NC_CACHE:
        _NC_CACHE["nc"] = _build_nc()
    nc = _NC_CACHE["nc"]

    in_maps = _host_inputs(
        x, np.asarray(rope_cos, np.float32), np.asarray(rope_sin, np.float32),
        W_qkv, b_qkv, W_proj, b_proj, num_special,
    )
    trace = bool(int(os.environ.get("KERNEL_TRACE", "0")))
    res = run_bass_kernel_spmd(nc, in_maps, core_ids=list(range(8)), trace=trace)
    LAST_RESULTS = res

    bias = b_proj + b_qkv[2 * 768 :] @ W_proj
    out = np.empty((B, N, C), np.float32)
    for b in range(B):
        out[b] = (res.results[2 * b]["y"].astype(np.float32)
                  + res.results[2 * b + 1]["y"].astype(np.float32) + bias)
    return out


# revision 48
# speedup vs baseline: 1.2083x; 1.0288x over previous
"""Trainium2 Bass kernel for nn_Attention_49134425866421 (v3).

Dense transformer attention block:
  qkv = x @ W_qkv + b_qkv -> partial RoPE on q,k -> softmax attention -> out proj.

Shapes (hardcoded): B=4, N=2048, C=768, H=12, D=64, fp32 in/out.

Sharding: 8 cores = (batch b in 0..3) x (head-group g in 0..1, 6 heads each).
Host sums the two partials per batch and adds b_proj + b_v @ W_proj (softmax
rows sum to 1, so the V bias contributes exactly b_v @ W_proj - host side).

Design (vs 304us v1 baseline):
 - all-bf16 datapath; qk bias folded into the projection matmul as a 7th
   accumulation step (ones-row x bias-row), so PSUM evacuation is a plain
   ACT/DVE copy and rope is 2 DVE muls + 1 DVE stream_shuffle + 1 Pool add.
 - RoPE rotate_half via stream_shuffle: head-dim layout permuted host-side
   (P64) so rotate pairs sit 16 partitions apart within one 32-block.
 - attention in two global q-half passes (q 0:1024, 1024:2048). Per (pass, h,
   jt): 2 score matmuls -> [128,1024] PSUM (2 banks, double-buffered) -> one
   1024-wide exp -> 8 transposed-AV matmuls (free dim 65, accumulating
   [q128, d+rowsum] over jt; rowsum rides V's ones column).
 - per-(h,pass) chain: reciprocal + per-subtile normalize (per-partition
   scalar), PE transpose back to head-major attnT. Chain pieces and deferred
   work (V tiles 4..15, qk pairs 1,2) are drip-fed one piece per attention
   step into PE/DVE slack.
 - first-half projection runs during pass 2; only the second half's
   projection is tail-exposed. y is bf16, summed on host.
PSUM: st 2x2 banks + acc 2 + trp 1 + pj 1 = 8.
"""

import os
import sys

import numpy as np

try:
    import concourse.bass as bass  # noqa: F401
except ImportError:
    sys.path.insert(0, "/opt/trn_rl_repo")

import ml_dtypes

B, N, C, H, D = 4, 2048, 768, 12, 64
HPC = 6          # heads per core
NPAIR = 3
P = 128
NT = N // P      # 16 token tiles
TC = 512
NTC = N // TC    # 4
QH = 1024        # q-half width
NSUB = 8         # q-subtiles per pass

P64 = np.concatenate([np.arange(0, 16), np.arange(32, 48),
                      np.arange(16, 32), np.arange(48, 64)])
SHUF_MASK = [(i + 16) % 32 for i in range(32)]

_NC_CACHE = {}
LAST_RESULTS = None


def _build_nc():
    from contextlib import ExitStack

    import concourse.bass as bass
    import concourse.bacc as bacc
    import concourse.mybir as mybir
    import concourse.tile as tile

    f32 = mybir.dt.float32
    bf16 = mybir.dt.bfloat16
    EXP = mybir.ActivationFunctionType.Exp
    IDENT = mybir.ActivationFunctionType.Identity

    nc = bacc.Bacc(None, target_bir_lowering=False)

    xT_d = nc.dram_tensor("xT", [C, N], bf16, kind="ExternalInput")
    wqk_d = nc.dram_tensor("w_qk", [P, 6, 768], bf16, kind="ExternalInput")
    wv_d = nc.dram_tensor("w_v", [P, 6, 384], bf16, kind="ExternalInput")
    wp_d = nc.dram_tensor("w_p", [P, 3, 768], bf16, kind="ExternalInput")
    bqkt_d = nc.dram_tensor("b_qk_t", [P, 6], f32, kind="ExternalInput")
    cos_d = nc.dram_tensor("cos_tab", [P, N], bf16, kind="ExternalInput")
    m2s_d = nc.dram_tensor("m2s_tab", [P, N], bf16, kind="ExternalInput")
    ident_d = nc.dram_tensor("ident", [P, P], bf16, kind="ExternalInput")
    y_d = nc.dram_tensor("y", [N, C], bf16, kind="ExternalOutput")
    debug = bool(int(os.environ.get("KERNEL_DEBUG", "0")))
    if debug:
        dbg_q = nc.dram_tensor("dbg_q", [P, NPAIR, N], bf16, kind="ExternalOutput")
        dbg_k = nc.dram_tensor("dbg_k", [P, NPAIR, N], bf16, kind="ExternalOutput")
        dbg_v = nc.dram_tensor("dbg_v", [P, NT, HPC, D + 1], bf16,
                               kind="ExternalOutput")
        dbg_a = nc.dram_tensor("dbg_a", [P, NPAIR, N], bf16, kind="ExternalOutput")

    xT_r = xT_d.rearrange("(ko p) t -> p ko t", p=P)

    with tile.TileContext(nc) as tc, ExitStack() as ctx:
        singles = ctx.enter_context(tc.tile_pool(name="singles", bufs=1))
        rope_p = ctx.enter_context(tc.tile_pool(name="rope", bufs=2))
        pt_p = ctx.enter_context(tc.tile_pool(name="ptp", bufs=36))
        an_p = ctx.enter_context(tc.tile_pool(name="anp", bufs=10))
        yt_p = ctx.enter_context(tc.tile_pool(name="ytp", bufs=4))
        rec_p = ctx.enter_context(tc.tile_pool(name="recp", bufs=2))
        st_p = ctx.enter_context(tc.tile_pool(name="stp", bufs=2, space="PSUM"))
        acc_p = ctx.enter_context(tc.tile_pool(name="accp", bufs=1, space="PSUM"))
        aux_p = ctx.enter_context(tc.tile_pool(name="auxp", bufs=1, space="PSUM"))
        pj_p = ctx.enter_context(tc.tile_pool(name="pjp", bufs=1, space="PSUM"))

        xT = singles.tile([P, 6, N], bf16)
        wqk = singles.tile([P, 6, 768], bf16)
        wv = singles.tile([P, 6, 384], bf16)
        wp = singles.tile([P, 3, 768], bf16)
        bqkt = singles.tile([P, 6], f32)
        cosT = singles.tile([P, N], bf16)
        m2sT = singles.tile([P, N], bf16)
        ident = singles.tile([P, P], bf16)
        qT = singles.tile([P, NPAIR, N], bf16)
        kT = singles.tile([P, NPAIR, N], bf16)
        Vt = singles.tile([P, NT, HPC, D + 1], bf16)
        attnT = singles.tile([P, NPAIR, N], bf16)

        # ---- input DMAs: wqk on ACT queue; xT+tables chunk-interleaved on SP
        # so device-FIFO arrival matches phase-A consumption order ----
        nc.scalar.dma_start(bqkt[:], bqkt_d[:])
        for ko in range(6):
            nc.scalar.dma_start(wqk[:, ko, :], wqk_d[:, ko, :])
        for tcu in range(NTC):
            tsl = slice(tcu * TC, (tcu + 1) * TC)
            for ko in range(6):
                nc.sync.dma_start(xT[:, ko, tsl], xT_r[:, ko, tsl])
            nc.sync.dma_start(cosT[:, tsl], cos_d[:, tsl])
            nc.sync.dma_start(m2sT[:, tsl], m2s_d[:, tsl])
        nc.sync.dma_start(wv[:], wv_d[:])
        nc.sync.dma_start(ident[:], ident_d[:])
        nc.sync.dma_start(wp[:], wp_d[:])

        nc.gpsimd.memset(Vt[:], 1.0)

        # p-state warmup: keep the PE busy from ~0.5us so it reaches full
        # clock (3us ramp) before the real matmuls arrive
        wps = pj_p.tile([P, TC], f32, tag="pj", name="wps")
        for _ in range(40):
            nc.tensor.matmul(wps[:6, :6], lhsT=bqkt[:1, :6], rhs=bqkt[:1, :6],
                             start=True, stop=True)

        def qk_mms(ps, mt, tsl):
            for ko in range(6):
                nc.tensor.matmul(
                    ps, lhsT=wqk[:, ko, mt * P : (mt + 1) * P],
                    rhs=xT[:, ko, tsl], start=(ko == 0), stop=(ko == 5))

        def rope_chain(ps, mt, tsl, dst, pb_engine):
            # PSUM evacuation + per-partition qk bias in one op
            pb = rope_p.tile([P, TC], bf16, tag="pb", bufs=12)
            if pb_engine == "act":
                nc.scalar.activation(pb[:], ps[:], IDENT,
                                     bias=bqkt[:, mt : mt + 1])
            else:
                nc.vector.tensor_scalar_add(out=pb[:], in0=ps[:],
                                            scalar1=bqkt[:, mt : mt + 1])
            qs = rope_p.tile([P, TC], bf16, tag="qs")
            qsw = rope_p.tile([P, TC], bf16, tag="qsw")
            nc.vector.tensor_mul(out=qs[:], in0=pb[:], in1=m2sT[:, tsl])
            nc.vector.tensor_mul(out=dst[:], in0=pb[:], in1=cosT[:, tsl])
            nc.vector.stream_shuffle(qsw[:], qs[:], SHUF_MASK)
            nc.gpsimd.tensor_add(out=dst[:], in0=dst[:], in1=qsw[:])

        def v_mms(ps, jt, ko_range):
            for ko in ko_range:
                nc.tensor.matmul(
                    ps[:, :384], lhsT=xT[:, ko, jt * P : (jt + 1) * P],
                    rhs=wv[:, ko, :], start=(ko == 0), stop=(ko == 5))

        def v_evac(ps, jt, engine):
            out = Vt[:, jt, :, : D]
            src = ps[:, :384].rearrange("p (h d) -> p h d", h=HPC)
            if engine == "act":
                nc.scalar.copy(out=out, in_=src)
            else:
                nc.vector.tensor_copy(out=out, in_=src)

        # ======== phase A: qk pairs 0,1 + V[12..15] ========
        for tcu in range(NTC):
            tsl = slice(tcu * TC, (tcu + 1) * TC)
            for mt in (3, 0, 4, 1):  # pair0 k, q; pair1 k, q
                dst = qT if mt < 3 else kT
                hp = mt if mt < 3 else mt - 3
                ps = st_p.tile([P, QH], f32, tag="st", name="psA")
                qk_mms(ps[:, :TC], mt, tsl)
                rope_chain(ps[:, :TC], mt, tsl, dst[:, hp, tsl], "act")


        # =================== deferred-work pieces ===================
        urgent = []
        lazy = []

        dp_ctr = [0]

        def defer_ps():
            # alternate deferred-piece PSUM between the pj and aux banks so
            # two pieces are in flight and the PE never waits on a bank free
            # (aux shares its slot with the chain trp tiles via the same tag)
            dp_ctr[0] += 1
            if dp_ctr[0] % 2:
                return pj_p.tile([P, TC], f32, tag="pj", name="dps")
            return aux_p.tile([P, TC], f32, tag="trp", name="dps")

        def defer_v(jt):
            state = {}
            def part1():
                ps = defer_ps()
                state["ps"] = ps
                for ko in range(3):
                    nc.tensor.matmul(
                        ps[:, :384], lhsT=xT[:, ko, jt * P : (jt + 1) * P],
                        rhs=wv[:, ko, :], start=(ko == 0), stop=False)
            def part2():
                ps = state["ps"]
                for ko in range(3, 6):
                    nc.tensor.matmul(
                        ps[:, :384], lhsT=xT[:, ko, jt * P : (jt + 1) * P],
                        rhs=wv[:, ko, :], start=False, stop=(ko == 5))
                v_evac(ps, jt, "dve")
            urgent.append((480, part1))
            urgent.append((480, part2))

        def defer_qk(mt, tcu):
            tsl = slice(tcu * TC, (tcu + 1) * TC)
            dst = qT if mt < 3 else kT
            hp = mt if mt < 3 else mt - 3
            state = {}
            def part1():
                ps = defer_ps()
                state["ps"] = ps
                for ko in range(3):
                    nc.tensor.matmul(
                        ps, lhsT=wqk[:, ko, mt * P : (mt + 1) * P],
                        rhs=xT[:, ko, tsl], start=(ko == 0), stop=False)
            def part2():
                ps = state["ps"]
                for ko in range(3, 6):
                    nc.tensor.matmul(
                        ps, lhsT=wqk[:, ko, mt * P : (mt + 1) * P],
                        rhs=xT[:, ko, tsl], start=False, stop=(ko == 5))
                rope_chain(ps, mt, tsl, dst[:, hp, tsl], "dve")
            lazy.append((640, part1))
            lazy.append((640, part2))

        for jt in range(NT):
            defer_v(jt)
        for mt in (5, 2):       # pair 2 k, q
            for tcu in range(NTC):
                defer_qk(mt, tcu)

        def proj_piece(tt, ch, pool, stage_engine):
            def piece():
                if pool is pj_p:
                    ps = pj_p.tile([P, TC], f32, tag="pj", name="pjps")
                else:
                    ps = st_p.tile([P, QH], f32, tag="st", name="pjst")
                for ko in range(3):
                    nc.tensor.matmul(
                        ps[:, :384],
                        lhsT=attnT[:, ko, tt * P : (tt + 1) * P],
                        rhs=wp[:, ko, ch * 384 : (ch + 1) * 384],
                        start=(ko == 0), stop=(ko == 2))
                yt = yt_p.tile([P, 384], bf16, tag="yt")
                if stage_engine == "act":
                    nc.scalar.copy(out=yt[:], in_=ps[:, :384])
                else:
                    nc.vector.tensor_copy(out=yt[:], in_=ps[:, :384])
                nc.sync.dma_start(
                    out=y_d[tt * P : (tt + 1) * P, ch * 384 : (ch + 1) * 384],
                    in_=yt[:])
            return piece

        # ============== attention: two q-half passes ==============
        # PSUM accumulation groups zero a whole 2KB bank on start, so the 8
        # AV accumulations per (head, pass) run SEQUENTIALLY (subtile-outer,
        # jt-inner) over the two acc banks. The jt loop emits scores+exp only
        # (keeping all 16 pt tiles); AV groups + normalize/transpose chains
        # are drip-fed into the next head's steps.
        def av_group(h, i, accs, pts, three_way=False):
            def piece():
                if three_way and i % 3 == 2:
                    acc = pj_p.tile([P, D + 1], f32, tag="pj", name="acc")
                elif three_way:
                    acc = acc_p.tile([P, D + 1], f32, tag=f"a{i % 3}", name="acc")
                else:
                    acc = acc_p.tile([P, D + 1], f32, tag=f"a{i % 2}", name="acc")
                accs[i] = acc
                for jt in range(NT):
                    nc.tensor.matmul(
                        acc[:], lhsT=pts[jt][:, i * P : (i + 1) * P],
                        rhs=Vt[:, jt, h, :],
                        start=(jt == 0), stop=(jt == NT - 1))
            return piece

        def chain_piece(h, psx, i, accs):
            hp, base = h // 2, 64 * (h % 2)
            tt = psx * NSUB + i
            def piece():
                acc = accs[i]
                rec = rec_p.tile([P, 1], f32, tag="rec", name="rec")
                nc.vector.reciprocal(out=rec[:], in_=acc[:, D : D + 1])
                anorm = an_p.tile([P, D], bf16, tag="an", name="anorm")
                nc.vector.tensor_scalar_mul(
                    out=anorm[:], in0=acc[:, :D], scalar1=rec[:])
                trp = aux_p.tile([P, P], bf16, tag="trp", name="trp")
                nc.tensor.transpose(
                    trp[base : base + D, :], anorm[:], ident[:],
                    tile_position=(0, base))
                nc.vector.tensor_copy(
                    out=attnT[base : base + D, hp, tt * P : (tt + 1) * P],
                    in_=trp[base : base + D, :])
            return piece

        for psx in range(2):
            for h in range(HPC):
                hp, base = h // 2, 64 * (h % 2)
                pts = []
                for jt in range(NT):
                    st = st_p.tile([P, QH], f32, tag="st", name="st")
                    for qc in range(2):
                        q0 = psx * QH + qc * TC
                        nc.tensor.matmul(
                            st[:, qc * TC : (qc + 1) * TC],
                            lhsT=kT[base : base + D, hp, jt * P : (jt + 1) * P],
                            rhs=qT[base : base + D, hp, q0 : q0 + TC],
                            start=True, stop=True,
                            tile_position=(base, 0))
                    pt = pt_p.tile([P, QH], bf16, tag="pt")
                    nc.scalar.activation(pt[:], st[:], EXP, scale=0.125)
                    pts.append(pt)
                    spent = 0
                    while urgent and spent < 450:
                        c, fn = urgent.pop(0)
                        fn()
                        spent += c
                    while lazy and spent < 450:
                        c, fn = lazy.pop(0)
                        fn()
                        spent += c
                accs = {}
                if psx == 1 and h == HPC - 1:
                    # drain spilled pieces BEFORE claiming the acc banks for
                    # the tail-head groups (rotation must match program order)
                    while urgent:
                        urgent.pop(0)[1]()
                    av_group(h, 0, accs, pts, three_way=True)()
                    av_group(h, 1, accs, pts, three_way=True)()
                    tail_work = (h, psx, accs, pts)
                else:
                    for i in range(NSUB):
                        urgent.append((432, av_group(h, i, accs, pts)))
                        urgent.append((100, chain_piece(h, psx, i, accs)))
                if psx == 1 and h == 1:
                    # first-half projection: attnT[0:8 tiles] complete
                    for tt in range(NSUB):
                        for ch in range(2):
                            lazy.append((480, proj_piece(tt, ch, pj_p, "dve")))

        # ---- tail: h5/pass2 AV groups 2..7 + chain + 2nd-half projection;
        # groups 0,1 already ran inline, so chain/proj for them start at once
        # while group i+2 accumulates in the bank freed by chain i ----
        h, psx, accs, pts = tail_work
        av_group(h, 2, accs, pts, three_way=True)()
        chain_piece(h, psx, 0, accs)()
        for i in range(NSUB):
            if i + 1 < NSUB:
                chain_piece(h, psx, i + 1, accs)()
            if i + 3 < NSUB:
                av_group(h, i + 3, accs, pts, three_way=True)()
            tt = NSUB + i
            pj = st_p.tile([P, QH], f32, tag="st", name="pjt")
            for ch in range(2):
                for ko in range(3):
                    nc.tensor.matmul(
                        pj[:, ch * TC : ch * TC + 384],
                        lhsT=attnT[:, ko, tt * P : (tt + 1) * P],
                        rhs=wp[:, ko, ch * 384 : (ch + 1) * 384],
                        start=(ko == 0), stop=(ko == 2))
            yt = yt_p.tile([P, 768], bf16, tag="yt2", name="yt2")
            src_ap = pj[:, : 2 * TC].rearrange("p (b x) -> p b x", b=2)[:, :, :384]
            eng = nc.scalar if i % 2 else nc.vector
            if i % 2:
                nc.scalar.copy(out=yt.rearrange("p (b x) -> p b x", b=2), in_=src_ap)
            else:
                nc.vector.tensor_copy(out=yt.rearrange("p (b x) -> p b x", b=2),
                                      in_=src_ap)
            nc.sync.dma_start(out=y_d[tt * P : (tt + 1) * P, :], in_=yt[:])
        while lazy:
            lazy.pop(0)[1]()

        if debug:
            nc.sync.dma_start(dbg_q[:], qT[:])
            nc.sync.dma_start(dbg_k[:], kT[:])
            nc.sync.dma_start(dbg_v[:], Vt[:])
            nc.sync.dma_start(dbg_a[:], attnT[:])

    nc.finalize()
    return nc


def _host_inputs(x, rope_cos, rope_sin, W_qkv, b_qkv, W_proj, b_proj, num_special):
    ns = int(num_special)
    bf = ml_dtypes.bfloat16
    cos_pad = np.ones((N, D), np.float32)
    sin_pad = np.zeros((N, D), np.float32)
    cos_pad[ns:] = rope_cos
    sin_pad[ns:] = rope_sin
    m2s = np.empty_like(sin_pad)
    m2s[:, : D // 2] = sin_pad[:, D // 2 :]
    m2s[:, D // 2 :] = -sin_pad[:, : D // 2]
    cos_tab = np.tile(np.ascontiguousarray(cos_pad.T[P64]), (2, 1)).astype(bf)
    m2s_tab = np.tile(np.ascontiguousarray(m2s.T[P64]), (2, 1)).astype(bf)
    ident = np.eye(P, dtype=np.float32).astype(bf)

    in_maps = []
    for core in range(8):
        b, g = core // 2, core % 2
        hs = list(range(HPC * g, HPC * g + HPC))
        cols_qk = []
        for mt in range(6):
            s, hp = (0, mt) if mt < 3 else (1, mt - 3)
            for half in range(2):
                h = hs[2 * hp + half]
                cols_qk.extend(s * 768 + h * 64 + int(P64[p]) for p in range(D))
        cols_qk = np.array(cols_qk)
        cols_v = np.array([2 * 768 + hs[i // 64] * 64 + (i % 64) for i in range(384)])
        rows_p = np.array(
            [hs[2 * ko + half] * 64 + d
             for ko in range(3) for half in range(2) for d in range(D)]
        )
        in_maps.append({
            "xT": np.ascontiguousarray(x[b].T).astype(bf),
            "w_qk": np.ascontiguousarray(
                W_qkv[:, cols_qk].reshape(6, P, 768).transpose(1, 0, 2)).astype(bf),
            "w_v": np.ascontiguousarray(
                W_qkv[:, cols_v].reshape(6, P, 384).transpose(1, 0, 2)).astype(bf),
            "w_p": np.ascontiguousarray(
                W_proj[rows_p].reshape(3, P, 768).transpose(1, 0, 2)).astype(bf),
            "b_qk_t": np.ascontiguousarray(b_qkv[cols_qk].reshape(6, P).T),
            "cos_tab": cos_tab,
            "m2s_tab": m2s_tab,
            "ident": ident,
        })
    return in_maps


def kernel(x, rope_cos, rope_sin, W_qkv, b_qkv, W_proj, b_proj, num_special):
    global LAST_RESULTS
    from concourse.bass_utils import run_bass_kernel_spmd

    x = np.asarray(x, np.float32)
    W_qkv = np.asarray(W_qkv, np.float32)
    b_qkv = np.asarray(b_qkv, np.float32)
    W_proj = np.asarray(W_proj, np.float32)
    b_proj = np.asarray(b_proj, np.float32)
    if "nc" not in _NC_CACHE:
        _NC_CACHE["nc"] = _build_nc()
    nc = _NC_CACHE["nc"]

    in_maps = _host_inputs(
        x, np.asarray(rope_cos, np.float32), np.asarray(rope_sin, np.float32),
        W_qkv, b_qkv, W_proj, b_proj, num_special,
    )
    trace = bool(int(os.environ.get("KERNEL_TRACE", "0")))
    res = run_bass_kernel_spmd(nc, in_maps, core_ids=list(range(8)), trace=trace)
    LAST_RESULTS = res

    bias = b_proj + b_qkv[2 * 768 :] @ W_proj
    out = np.empty((B, N, C), np.float32)
    for b in range(B):
        out[b] = (res.results[2 * b]["y"].astype(np.float32)
                  + res.results[2 * b + 1]["y"].astype(np.float32) + bias)
    return out


# revision 52
# speedup vs baseline: 1.2304x; 1.0183x over previous
"""Trainium2 Bass kernel for nn_Attention_49134425866421 (v3).

Dense transformer attention block:
  qkv = x @ W_qkv + b_qkv -> partial RoPE on q,k -> softmax attention -> out proj.

Shapes (hardcoded): B=4, N=2048, C=768, H=12, D=64, fp32 in/out.

Sharding: 8 cores = (batch b in 0..3) x (head-group g in 0..1, 6 heads each).
Host sums the two partials per batch and adds b_proj + b_v @ W_proj (softmax
rows sum to 1, so the V bias contributes exactly b_v @ W_proj - host side).

Design (vs 304us v1 baseline):
 - all-bf16 datapath; qk bias folded into the projection matmul as a 7th
   accumulation step (ones-row x bias-row), so PSUM evacuation is a plain
   ACT/DVE copy and rope is 2 DVE muls + 1 DVE stream_shuffle + 1 Pool add.
 - RoPE rotate_half via stream_shuffle: head-dim layout permuted host-side
   (P64) so rotate pairs sit 16 partitions apart within one 32-block.
 - attention in two global q-half passes (q 0:1024, 1024:2048). Per (pass, h,
   jt): 2 score matmuls -> [128,1024] PSUM (2 banks, double-buffered) -> one
   1024-wide exp -> 8 transposed-AV matmuls (free dim 65, accumulating
   [q128, d+rowsum] over jt; rowsum rides V's ones column).
 - per-(h,pass) chain: reciprocal + per-subtile normalize (per-partition
   scalar), PE transpose back to head-major attnT. Chain pieces and deferred
   work (V tiles 4..15, qk pairs 1,2) are drip-fed one piece per attention
   step into PE/DVE slack.
 - first-half projection runs during pass 2; only the second half's
   projection is tail-exposed. y is bf16, summed on host.
PSUM: st 2x2 banks + acc 2 + trp 1 + pj 1 = 8.
"""

import os
import sys

import numpy as np

try:
    import concourse.bass as bass  # noqa: F401
except ImportError:
    sys.path.insert(0, "/opt/trn_rl_repo")

import ml_dtypes

B, N, C, H, D = 4, 2048, 768, 12, 64
HPC = 6          # heads per core
NPAIR = 3
P = 128
NT = N // P      # 16 token tiles
TC = 512
NTC = N // TC    # 4
QH = 1024        # q-half width
NSUB = 8         # q-subtiles per pass

P64 = np.concatenate([np.arange(0, 16), np.arange(32, 48),
                      np.arange(16, 32), np.arange(48, 64)])
SHUF_MASK = [(i + 16) % 32 for i in range(32)]

_NC_CACHE = {}
LAST_RESULTS = None


def _build_nc():
    from contextlib import ExitStack

    import concourse.bass as bass
    import concourse.bacc as bacc
    import concourse.mybir as mybir
    import concourse.tile as tile

    f32 = mybir.dt.float32
    bf16 = mybir.dt.bfloat16
    EXP = mybir.ActivationFunctionType.Exp
    IDENT = mybir.ActivationFunctionType.Identity

    nc = bacc.Bacc(None, target_bir_lowering=False)

    xT_d = nc.dram_tensor("xT", [C, N], bf16, kind="ExternalInput")
    wqk_d = nc.dram_tensor("w_qk", [P, 6, 768], bf16, kind="ExternalInput")
    wv_d = nc.dram_tensor("w_v", [P, 6, 384], bf16, kind="ExternalInput")
    wp_d = nc.dram_tensor("w_p", [P, 3, 768], bf16, kind="ExternalInput")
    bqkt_d = nc.dram_tensor("b_qk_t", [P, 6], f32, kind="ExternalInput")
    cos_d = nc.dram_tensor("cos_tab", [P, N], bf16, kind="ExternalInput")
    m2s_d = nc.dram_tensor("m2s_tab", [P, N], bf16, kind="ExternalInput")
    ident_d = nc.dram_tensor("ident", [P, P], bf16, kind="ExternalInput")
    y_d = nc.dram_tensor("y", [N, C], bf16, kind="ExternalOutput")
    debug = bool(int(os.environ.get("KERNEL_DEBUG", "0")))
    if debug:
        dbg_q = nc.dram_tensor("dbg_q", [P, NPAIR, N], bf16, kind="ExternalOutput")
        dbg_k = nc.dram_tensor("dbg_k", [P, NPAIR, N], bf16, kind="ExternalOutput")
        dbg_v = nc.dram_tensor("dbg_v", [P, NT, HPC, D + 1], bf16,
                               kind="ExternalOutput")
        dbg_a = nc.dram_tensor("dbg_a", [P, NPAIR, N], bf16, kind="ExternalOutput")

    xT_r = xT_d.rearrange("(ko p) t -> p ko t", p=P)

    with tile.TileContext(nc) as tc, ExitStack() as ctx:
        singles = ctx.enter_context(tc.tile_pool(name="singles", bufs=1))
        rope_p = ctx.enter_context(tc.tile_pool(name="rope", bufs=2))
        pt_p = ctx.enter_context(tc.tile_pool(name="ptp", bufs=36))
        an_p = ctx.enter_context(tc.tile_pool(name="anp", bufs=10))
        yt_p = ctx.enter_context(tc.tile_pool(name="ytp", bufs=4))
        rec_p = ctx.enter_context(tc.tile_pool(name="recp", bufs=2))
        st_p = ctx.enter_context(tc.tile_pool(name="stp", bufs=2, space="PSUM"))
        acc_p = ctx.enter_context(tc.tile_pool(name="accp", bufs=1, space="PSUM"))
        aux_p = ctx.enter_context(tc.tile_pool(name="auxp", bufs=1, space="PSUM"))
        pj_p = ctx.enter_context(tc.tile_pool(name="pjp", bufs=1, space="PSUM"))

        xT = singles.tile([P, 6, N], bf16)
        wqk = singles.tile([P, 6, 768], bf16)
        wv = singles.tile([P, 6, 384], bf16)
        wp = singles.tile([P, 3, 768], bf16)
        bqkt = singles.tile([P, 6], f32)
        cosT = singles.tile([P, N], bf16)
        m2sT = singles.tile([P, N], bf16)
        ident = singles.tile([P, P], bf16)
        qT = singles.tile([P, NPAIR, N], bf16)
        kT = singles.tile([P, NPAIR, N], bf16)
        Vt = singles.tile([P, NT, HPC, D + 1], bf16)
        attnT = singles.tile([P, NPAIR, N], bf16)

        # ---- input DMAs: wqk on ACT queue; xT+tables chunk-interleaved on SP
        # so device-FIFO arrival matches phase-A consumption order ----
        nc.scalar.dma_start(bqkt[:], bqkt_d[:])
        for ko in range(6):
            nc.scalar.dma_start(wqk[:, ko, :], wqk_d[:, ko, :])
        for tcu in range(NTC):
            tsl = slice(tcu * TC, (tcu + 1) * TC)
            for ko in range(6):
                nc.sync.dma_start(xT[:, ko, tsl], xT_r[:, ko, tsl])
            nc.sync.dma_start(cosT[:, tsl], cos_d[:, tsl])
            nc.sync.dma_start(m2sT[:, tsl], m2s_d[:, tsl])
        nc.sync.dma_start(wv[:], wv_d[:])
        nc.sync.dma_start(ident[:], ident_d[:])
        nc.sync.dma_start(wp[:], wp_d[:])

        nc.gpsimd.memset(Vt[:], 1.0)

        # p-state warmup: keep the PE busy from ~0.5us so it reaches full
        # clock (3us ramp) before the real matmuls arrive
        wps = pj_p.tile([P, TC], f32, tag="pj", name="wps")
        for _ in range(40):
            nc.tensor.matmul(wps[:6, :6], lhsT=bqkt[:1, :6], rhs=bqkt[:1, :6],
                             start=True, stop=True)

        def qk_mms(ps, mt, tsl):
            for ko in range(6):
                nc.tensor.matmul(
                    ps, lhsT=wqk[:, ko, mt * P : (mt + 1) * P],
                    rhs=xT[:, ko, tsl], start=(ko == 0), stop=(ko == 5))

        def rope_chain(ps, mt, tsl, dst, pb_engine):
            # PSUM evacuation + per-partition qk bias in one op
            pb = rope_p.tile([P, TC], bf16, tag="pb", bufs=12)
            if pb_engine == "act":
                nc.scalar.activation(pb[:], ps[:], IDENT,
                                     bias=bqkt[:, mt : mt + 1])
            else:
                nc.vector.tensor_scalar_add(out=pb[:], in0=ps[:],
                                            scalar1=bqkt[:, mt : mt + 1])
            qs = rope_p.tile([P, TC], bf16, tag="qs")
            qsw = rope_p.tile([P, TC], bf16, tag="qsw")
            nc.vector.tensor_mul(out=qs[:], in0=pb[:], in1=m2sT[:, tsl])
            nc.vector.tensor_mul(out=dst[:], in0=pb[:], in1=cosT[:, tsl])
            nc.vector.stream_shuffle(qsw[:], qs[:], SHUF_MASK)
            nc.gpsimd.tensor_add(out=dst[:], in0=dst[:], in1=qsw[:])

        def v_mms(ps, jt, ko_range):
            for ko in ko_range:
                nc.tensor.matmul(
                    ps[:, :384], lhsT=xT[:, ko, jt * P : (jt + 1) * P],
                    rhs=wv[:, ko, :], start=(ko == 0), stop=(ko == 5))

        def v_evac(ps, jt, engine):
            out = Vt[:, jt, :, : D]
            src = ps[:, :384].rearrange("p (h d) -> p h d", h=HPC)
            if engine == "act":
                nc.scalar.copy(out=out, in_=src)
            else:
                nc.vector.tensor_copy(out=out, in_=src)

        # ==== phase A (minimal): pair-0 k all chunks + q first half ====
        # first scores only need kT pair0 (k-tiles stream jt=0..15) and
        # qT pair0 q<1024; everything else is deadline-scheduled into
        # attention PE slack
        for tcu in range(NTC):
            tsl = slice(tcu * TC, (tcu + 1) * TC)
            mts = (3, 0) if tcu < 2 else (3,)
            for mt in mts:
                dst = qT if mt < 3 else kT
                hp = mt if mt < 3 else mt - 3
                ps = st_p.tile([P, QH], f32, tag="st", name="psA")
                qk_mms(ps[:, :TC], mt, tsl)
                rope_chain(ps[:, :TC], mt, tsl, dst[:, hp, tsl], "act")

        # =================== deferred-work pieces ===================
        # single deadline-ordered queue; each step pops pieces by earliest
        # deadline while a per-step PE-cost budget remains
        import heapq
        sched = []
        seqn = [0]

        def enq(deadline, cost, fn):
            heapq.heappush(sched, (deadline, seqn[0], cost, fn))
            seqn[0] += 1

        dp_ctr = [0]

        def defer_ps():
            dp_ctr[0] += 1
            if dp_ctr[0] % 2:
                return pj_p.tile([P, TC], f32, tag="pj", name="dps")
            return aux_p.tile([P, TC], f32, tag="trp", name="dps")

        def defer_v(jt, deadline):
            state = {}
            def part1():
                ps = defer_ps()
                state["ps"] = ps
                for ko in range(3):
                    nc.tensor.matmul(
                        ps[:, :384], lhsT=xT[:, ko, jt * P : (jt + 1) * P],
                        rhs=wv[:, ko, :], start=(ko == 0), stop=False)
            def part2():
                ps = state["ps"]
                for ko in range(3, 6):
                    nc.tensor.matmul(
                        ps[:, :384], lhsT=xT[:, ko, jt * P : (jt + 1) * P],
                        rhs=wv[:, ko, :], start=False, stop=(ko == 5))
                v_evac(ps, jt, "dve")
            enq(deadline, 480, part1)
            enq(deadline, 480, part2)

        def defer_qk(mt, tcu, deadline):
            tsl = slice(tcu * TC, (tcu + 1) * TC)
            dst = qT if mt < 3 else kT
            hp = mt if mt < 3 else mt - 3
            state = {}
            def part1():
                ps = defer_ps()
                state["ps"] = ps
                for ko in range(3):
                    nc.tensor.matmul(
                        ps, lhsT=wqk[:, ko, mt * P : (mt + 1) * P],
                        rhs=xT[:, ko, tsl], start=(ko == 0), stop=False)
            def part2():
                ps = state["ps"]
                for ko in range(3, 6):
                    nc.tensor.matmul(
                        ps, lhsT=wqk[:, ko, mt * P : (mt + 1) * P],
                        rhs=xT[:, ko, tsl], start=False, stop=(ko == 5))
                rope_chain(ps, mt, tsl, dst[:, hp, tsl], "dve")
            enq(deadline, 640, part1)
            enq(deadline, 640, part2)

        for tcu in (2, 3):              # pair0 q second half (pass-2 windows)
            defer_qk(0, tcu, 20)
        for jt in range(NT):            # V tiles: strictly before window-0's
            defer_v(jt, 10 + jt // 2)   # AV groups (deadline 24)
        for mt in (4, 1):               # pair 1 (windows 4,5 = step 64)
            for tcu in range(NTC):
                defer_qk(mt, tcu, 52)
        for mt in (5, 2):               # pair 2 (windows 8,9 = step 128)
            for tcu in range(NTC):
                defer_qk(mt, tcu, 112)

        def proj_piece(tt, ch, pool, stage_engine):
            def piece():
                if pool is pj_p:
                    ps = pj_p.tile([P, TC], f32, tag="pj", name="pjps")
                else:
                    ps = st_p.tile([P, QH], f32, tag="st", name="pjst")
                for ko in range(3):
                    nc.tensor.matmul(
                        ps[:, :384],
                        lhsT=attnT[:, ko, tt * P : (tt + 1) * P],
                        rhs=wp[:, ko, ch * 384 : (ch + 1) * 384],
                        start=(ko == 0), stop=(ko == 2))
                yt = yt_p.tile([P, 384], bf16, tag="yt")
                if stage_engine == "act":
                    nc.scalar.copy(out=yt[:], in_=ps[:, :384])
                else:
                    nc.vector.tensor_copy(out=yt[:], in_=ps[:, :384])
                nc.sync.dma_start(
                    out=y_d[tt * P : (tt + 1) * P, ch * 384 : (ch + 1) * 384],
                    in_=yt[:])
            return piece

        # ============== attention: two q-half passes ==============
        # PSUM accumulation groups zero a whole 2KB bank on start, so the 8
        # AV accumulations per (head, pass) run SEQUENTIALLY (subtile-outer,
        # jt-inner) over the two acc banks. The jt loop emits scores+exp only
        # (keeping all 16 pt tiles); AV groups + normalize/transpose chains
        # are drip-fed into the next head's steps.
        def av_group(h, i, accs, pts, three_way=False):
            def piece():
                if three_way and i % 3 == 2:
                    acc = pj_p.tile([P, D + 1], f32, tag="pj", name="acc")
                elif three_way:
                    acc = acc_p.tile([P, D + 1], f32, tag=f"a{i % 3}", name="acc")
                else:
                    acc = acc_p.tile([P, D + 1], f32, tag=f"a{i % 2}", name="acc")
                accs[i] = acc
                for jt in range(NT):
                    nc.tensor.matmul(
                        acc[:], lhsT=pts[jt][:, i * P : (i + 1) * P],
                        rhs=Vt[:, jt, h, :],
                        start=(jt == 0), stop=(jt == NT - 1))
            return piece

        def chain_piece(h, psx, i, accs):
            hp, base = h // 2, 64 * (h % 2)
            tt = psx * NSUB + i
            def piece():
                acc = accs[i]
                rec = rec_p.tile([P, 1], f32, tag="rec", name="rec")
                nc.vector.reciprocal(out=rec[:], in_=acc[:, D : D + 1])
                anorm = an_p.tile([P, D], bf16, tag="an", name="anorm")
                nc.vector.tensor_scalar_mul(
                    out=anorm[:], in0=acc[:, :D], scalar1=rec[:])
                trp = aux_p.tile([P, P], bf16, tag="trp", name="trp")
                nc.tensor.transpose(
                    trp[base : base + D, :], anorm[:], ident[:],
                    tile_position=(0, base))
                nc.vector.tensor_copy(
                    out=attnT[base : base + D, hp, tt * P : (tt + 1) * P],
                    in_=trp[base : base + D, :])
            return piece

        # window order interleaves the two q-half passes per head pair so
        # pair-1 qk is not needed until step 64 and pair-2 until step 128
        WINDOWS = [(0, 0), (0, 1), (1, 0), (1, 1), (0, 2), (0, 3),
                   (1, 2), (1, 3), (0, 4), (0, 5), (1, 4), (1, 5)]
        for w, (psx, h) in enumerate(WINDOWS):
            hp, base = h // 2, 64 * (h % 2)
            pts = []
            for jt in range(NT):
                st = st_p.tile([P, QH], f32, tag="st", name="st")
                for qc in range(2):
                    q0 = psx * QH + qc * TC
                    nc.tensor.matmul(
                        st[:, qc * TC : (qc + 1) * TC],
                        lhsT=kT[base : base + D, hp, jt * P : (jt + 1) * P],
                        rhs=qT[base : base + D, hp, q0 : q0 + TC],
                        start=True, stop=True,
                        tile_position=(base, 0))
                pt = pt_p.tile([P, QH], bf16, tag="pt")
                nc.scalar.activation(pt[:], st[:], EXP, scale=0.125)
                pts.append(pt)
                spent = 0
                while sched and spent < 450:
                    _, _, c, fn = heapq.heappop(sched)
                    fn()
                    spent += c
            accs = {}
            if w == len(WINDOWS) - 1:
                # drain remaining pieces BEFORE claiming the acc banks for
                # the tail-head groups (rotation must match program order)
                while sched:
                    heapq.heappop(sched)[3]()
                av_group(h, 0, accs, pts, three_way=True)()
                av_group(h, 1, accs, pts, three_way=True)()
                tail_work = (h, psx, accs, pts)
            else:
                dl = (w + 1) * NT + 8
                for i in range(NSUB):
                    enq(dl, 432, av_group(h, i, accs, pts))
                    enq(dl, 100, chain_piece(h, psx, i, accs))
            if w == 9:
                # pass-1 attnT (tts 0..7) complete once window 9's chains
                # drain; project them during the last two windows
                for tt in range(NSUB):
                    for ch in range(2):
                        enq(172, 480, proj_piece(tt, ch, pj_p, "dve"))

        # ---- tail: h5/pass2 AV groups 2..7 + chain + 2nd-half projection;
        # groups 0,1 already ran inline, so chain/proj for them start at once
        # while group i+2 accumulates in the bank freed by chain i ----
        h, psx, accs, pts = tail_work
        av_group(h, 2, accs, pts, three_way=True)()
        chain_piece(h, psx, 0, accs)()
        for i in range(NSUB):
            if i + 1 < NSUB:
                chain_piece(h, psx, i + 1, accs)()
            if i + 3 < NSUB:
                av_group(h, i + 3, accs, pts, three_way=True)()
            tt = NSUB + i
            pj = st_p.tile([P, QH], f32, tag="st", name="pjt")
            for ch in range(2):
                for ko in range(3):
                    nc.tensor.matmul(
                        pj[:, ch * TC : ch * TC + 384],
                        lhsT=attnT[:, ko, tt * P : (tt + 1) * P],
                        rhs=wp[:, ko, ch * 384 : (ch + 1) * 384],
                        start=(ko == 0), stop=(ko == 2))
            yt = yt_p.tile([P, 768], bf16, tag="yt2", name="yt2")
            src_ap = pj[:, : 2 * TC].rearrange("p (b x) -> p b x", b=2)[:, :, :384]
            eng = nc.scalar if i % 2 else nc.vector
            if i % 2:
                nc.scalar.copy(out=yt.rearrange("p (b x) -> p b x", b=2), in_=src_ap)
            else:
                nc.vector.tensor_copy(out=yt.rearrange("p (b x) -> p b x", b=2),
                                      in_=src_ap)
            nc.sync.dma_start(out=y_d[tt * P : (tt + 1) * P, :], in_=yt[:])
        while sched:
            heapq.heappop(sched)[3]()

        if debug:
            nc.sync.dma_start(dbg_q[:], qT[:])
            nc.sync.dma_start(dbg_k[:], kT[:])
            nc.sync.dma_start(dbg_v[:], Vt[:])
            nc.sync.dma_start(dbg_a[:], attnT[:])

    nc.finalize()
    return nc


def _host_inputs(x, rope_cos, rope_sin, W_qkv, b_qkv, W_proj, b_proj, num_special):
    ns = int(num_special)
    bf = ml_dtypes.bfloat16
    cos_pad = np.ones((N, D), np.float32)
    sin_pad = np.zeros((N, D), np.float32)
    cos_pad[ns:] = rope_cos
    sin_pad[ns:] = rope_sin
    m2s = np.empty_like(sin_pad)
    m2s[:, : D // 2] = sin_pad[:, D // 2 :]
    m2s[:, D // 2 :] = -sin_pad[:, : D // 2]
    cos_tab = np.tile(np.ascontiguousarray(cos_pad.T[P64]), (2, 1)).astype(bf)
    m2s_tab = np.tile(np.ascontiguousarray(m2s.T[P64]), (2, 1)).astype(bf)
    ident = np.eye(P, dtype=np.float32).astype(bf)

    in_maps = []
    for core in range(8):
        b, g = core // 2, core % 2
        hs = list(range(HPC * g, HPC * g + HPC))
        cols_qk = []
        for mt in range(6):
            s, hp = (0, mt) if mt < 3 else (1, mt - 3)
            for half in range(2):
                h = hs[2 * hp + half]
                cols_qk.extend(s * 768 + h * 64 + int(P64[p]) for p in range(D))
        cols_qk = np.array(cols_qk)
        cols_v = np.array([2 * 768 + hs[i // 64] * 64 + (i % 64) for i in range(384)])
        rows_p = np.array(
            [hs[2 * ko + half] * 64 + d
             for ko in range(3) for half in range(2) for d in range(D)]
        )
        in_maps.append({
            "xT": np.ascontiguousarray(x[b].T).astype(bf),
            "w_qk": np.ascontiguousarray(
                W_qkv[:, cols_qk].reshape(6, P, 768).transpose(1, 0, 2)).astype(bf),
            "w_v": np.ascontiguousarray(
                W_qkv[:, cols_v].reshape(6, P, 384).transpose(1, 0, 2)).astype(bf),
            "w_p": np.ascontiguousarray(
                W_proj[rows_p].reshape(3, P, 768).transpose(1, 0, 2)).astype(bf),
            "b_qk_t": np.ascontiguousarray(b_qkv[cols_qk].reshape(6, P).T),
            "cos_tab": cos_tab,
            "m2s_tab": m2s_tab,
            "ident": ident,
        })
    return in_maps


def kernel(x, rope_cos, rope_sin, W_qkv, b_qkv, W_proj, b_proj, num_special):
    global LAST_RESULTS
    from concourse.bass_utils import run_bass_kernel_spmd

    x = np.asarray(x, np.float32)
    W_qkv = np.asarray(W_qkv, np.float32)
    b_qkv = np.asarray(b_qkv, np.float32)
    W_proj = np.asarray(W_proj, np.float32)
    b_proj = np.asarray(b_proj, np.float32)
    if "nc" not in _NC_CACHE:
        _NC_CACHE["nc"] = _build_nc()
    nc = _NC_CACHE["nc"]

    in_maps = _host_inputs(
        x, np.asarray(rope_cos, np.float32), np.asarray(rope_sin, np.float32),
        W_qkv, b_qkv, W_proj, b_proj, num_special,
    )
    trace = bool(int(os.environ.get("KERNEL_TRACE", "0")))
    res = run_bass_kernel_spmd(nc, in_maps, core_ids=list(range(8)), trace=trace)
    LAST_RESULTS = res

    bias = b_proj + b_qkv[2 * 768 :] @ W_proj
    out = np.empty((B, N, C), np.float32)
    for b in range(B):
        out[b] = (res.results[2 * b]["y"].astype(np.float32)
                  + res.results[2 * b + 1]["y"].astype(np.float32) + bias)
    return out


# revision 67
# speedup vs baseline: 1.2390x; 1.0069x over previous
"""Trainium2 Bass kernel for nn_Attention_49134425866421 (v4, 246560ns).

Dense transformer attention block:
  qkv = x @ W_qkv + b_qkv -> partial RoPE on q,k -> softmax attention -> out proj.

Shapes (hardcoded): B=4, N=2048, C=768, H=12, D=64, fp32 in/out.

Sharding: 8 cores = (batch b in 0..3) x (head-group g in 0..1, 6 heads each).
Host sums the two partials per batch and adds b_proj + b_v @ W_proj (softmax
rows sum to 1, so the device-side V path drops its bias; its projection
contribution is the host-side constant b_v @ W_proj).

Design (vs 304us v1 baseline; ACT exp is the critical engine at 199us):
 - all-bf16 datapath; qk bias folded into the PSUM evacuation via an
   ACT Identity-with-bias (or DVE tensor_scalar_add) op.
 - RoPE rotate_half via DVE stream_shuffle: head-dim layout permuted
   host-side (P64) so rotate pairs sit 16 partitions apart within one
   32-partition block; sign folded into the m2s table.
 - attention as 12 windows interleaving the two q-half passes per head pair
   ([p0 ps1 h0,h1, p0 ps2 h0,h1, p1 ...]) so pair-1 qk projection is not
   needed until step 64 and pair-2 until step 128. Per (window, k-tile):
   2 score matmuls -> [128,1024] PSUM (2 banks, double-buffered) -> one
   1024-wide exp -> pt (bf16). AV runs TRANSPOSED (out[q128, d+rowsum],
   free dim 65) as 8 sequential per-subtile accumulation groups over the
   two acc banks (PSUM start=True zeroes a whole 2KB bank, so groups must
   never share a bank while open); rowsum rides V's ones column.
 - per-subtile chain: reciprocal + per-partition normalize, PE transpose
   (identity matmul, tile_position col offset for odd heads) back to
   head-major attnT.
 - all remaining work (V tiles, qk pairs 0b/1/2, AV groups, chains, early
   projections) is drip-fed into attention PE slack by a deadline-ordered
   scheduler with a per-step PE-cost budget (450ns). Projection is split:
   token tiles 0..7 fully + tiles 8..15 pair-0/1 partials (to y2) run
   during late windows; the tail only computes the pair-2 contribution.
PSUM: st 2x2 banks + acc 2 + aux(trp/deferred) 1 + pj(deferred/acc3) 1 = 8.
"""

import os
import sys

import numpy as np

try:
    import concourse.bass as bass  # noqa: F401
except ImportError:
    sys.path.insert(0, "/opt/trn_rl_repo")

import ml_dtypes

B, N, C, H, D = 4, 2048, 768, 12, 64
HPC = 6          # heads per core
NPAIR = 3
P = 128
NT = N // P      # 16 token tiles
TC = 512
NTC = N // TC    # 4
QH = 1024        # q-half width
NSUB = 8         # q-subtiles per pass

P64 = np.concatenate([np.arange(0, 16), np.arange(32, 48),
                      np.arange(16, 32), np.arange(48, 64)])
SHUF_MASK = [(i + 16) % 32 for i in range(32)]

_NC_CACHE = {}
LAST_RESULTS = None


def _build_nc():
    from contextlib import ExitStack

    import concourse.bass as bass
    import concourse.bacc as bacc
    import concourse.mybir as mybir
    import concourse.tile as tile

    f32 = mybir.dt.float32
    bf16 = mybir.dt.bfloat16
    EXP = mybir.ActivationFunctionType.Exp
    IDENT = mybir.ActivationFunctionType.Identity

    nc = bacc.Bacc(None, target_bir_lowering=False)

    xT_d = nc.dram_tensor("xT", [C, N], bf16, kind="ExternalInput")
    wqk_d = nc.dram_tensor("w_qk", [P, 6, 768], bf16, kind="ExternalInput")
    wv_d = nc.dram_tensor("w_v", [P, 6, 384], bf16, kind="ExternalInput")
    wp_d = nc.dram_tensor("w_p", [P, 3, 768], bf16, kind="ExternalInput")
    bqkt_d = nc.dram_tensor("b_qk_t", [P, 6], f32, kind="ExternalInput")
    cos_d = nc.dram_tensor("cos_tab", [P, N], bf16, kind="ExternalInput")
    m2s_d = nc.dram_tensor("m2s_tab", [P, N], bf16, kind="ExternalInput")
    ident_d = nc.dram_tensor("ident", [P, P], bf16, kind="ExternalInput")
    y_d = nc.dram_tensor("y", [N, C], bf16, kind="ExternalOutput")
    y2_d = nc.dram_tensor("y2", [N // 2, C], bf16, kind="ExternalOutput")
    debug = bool(int(os.environ.get("KERNEL_DEBUG", "0")))
    if debug:
        dbg_q = nc.dram_tensor("dbg_q", [P, NPAIR, N], bf16, kind="ExternalOutput")
        dbg_k = nc.dram_tensor("dbg_k", [P, NPAIR, N], bf16, kind="ExternalOutput")
        dbg_v = nc.dram_tensor("dbg_v", [P, NT, HPC, D + 1], bf16,
                               kind="ExternalOutput")
        dbg_a = nc.dram_tensor("dbg_a", [P, NPAIR, N], bf16, kind="ExternalOutput")

    xT_r = xT_d.rearrange("(ko p) t -> p ko t", p=P)

    with tile.TileContext(nc) as tc, ExitStack() as ctx:
        singles = ctx.enter_context(tc.tile_pool(name="singles", bufs=1))
        rope_p = ctx.enter_context(tc.tile_pool(name="rope", bufs=2))
        pt_p = ctx.enter_context(tc.tile_pool(name="ptp", bufs=36))
        an_p = ctx.enter_context(tc.tile_pool(name="anp", bufs=10))
        yt_p = ctx.enter_context(tc.tile_pool(name="ytp", bufs=4))
        rec_p = ctx.enter_context(tc.tile_pool(name="recp", bufs=2))
        st_p = ctx.enter_context(tc.tile_pool(name="stp", bufs=2, space="PSUM"))
        acc_p = ctx.enter_context(tc.tile_pool(name="accp", bufs=1, space="PSUM"))
        aux_p = ctx.enter_context(tc.tile_pool(name="auxp", bufs=1, space="PSUM"))
        pj_p = ctx.enter_context(tc.tile_pool(name="pjp", bufs=1, space="PSUM"))

        xT = singles.tile([P, 6, N], bf16)
        wqk = singles.tile([P, 6, 768], bf16)
        wv = singles.tile([P, 6, 384], bf16)
        wp = singles.tile([P, 3, 768], bf16)
        bqkt = singles.tile([P, 6], f32)
        cosT = singles.tile([P, N], bf16)
        m2sT = singles.tile([P, N], bf16)
        ident = singles.tile([P, P], bf16)
        qT = singles.tile([P, NPAIR, N], bf16)
        kT = singles.tile([P, NPAIR, N], bf16)
        Vt = singles.tile([P, NT, HPC, D + 1], bf16)
        attnT = singles.tile([P, NPAIR, N], bf16)

        # ---- input DMAs: wqk on ACT queue; xT+tables chunk-interleaved on SP
        # so device-FIFO arrival matches phase-A consumption order ----
        nc.scalar.dma_start(bqkt[:], bqkt_d[:])
        for ko in range(6):
            nc.scalar.dma_start(wqk[:, ko, :], wqk_d[:, ko, :])
        for tcu in range(NTC):
            tsl = slice(tcu * TC, (tcu + 1) * TC)
            for ko in range(6):
                nc.sync.dma_start(xT[:, ko, tsl], xT_r[:, ko, tsl])
            nc.sync.dma_start(cosT[:, tsl], cos_d[:, tsl])
            nc.sync.dma_start(m2sT[:, tsl], m2s_d[:, tsl])
        nc.sync.dma_start(wv[:], wv_d[:])
        nc.sync.dma_start(ident[:], ident_d[:])
        nc.sync.dma_start(wp[:], wp_d[:])

        nc.gpsimd.memset(Vt[:], 1.0)

        # p-state warmup: keep the PE busy from ~0.5us so it reaches full
        # clock (3us ramp) before the real matmuls arrive
        wps = pj_p.tile([P, TC], f32, tag="pj", name="wps")
        for _ in range(40):
            nc.tensor.matmul(wps[:6, :6], lhsT=bqkt[:1, :6], rhs=bqkt[:1, :6],
                             start=True, stop=True)

        def qk_mms(ps, mt, tsl):
            for ko in range(6):
                nc.tensor.matmul(
                    ps, lhsT=wqk[:, ko, mt * P : (mt + 1) * P],
                    rhs=xT[:, ko, tsl], start=(ko == 0), stop=(ko == 5))

        def rope_chain(ps, mt, tsl, dst, pb_engine):
            # PSUM evacuation + per-partition qk bias in one op
            pb = rope_p.tile([P, TC], bf16, tag="pb", bufs=12)
            if pb_engine == "act":
                nc.scalar.activation(pb[:], ps[:], IDENT,
                                     bias=bqkt[:, mt : mt + 1])
            else:
                nc.vector.tensor_scalar_add(out=pb[:], in0=ps[:],
                                            scalar1=bqkt[:, mt : mt + 1])
            qs = rope_p.tile([P, TC], bf16, tag="qs")
            qsw = rope_p.tile([P, TC], bf16, tag="qsw")
            nc.vector.tensor_mul(out=qs[:], in0=pb[:], in1=m2sT[:, tsl])
            nc.vector.tensor_mul(out=dst[:], in0=pb[:], in1=cosT[:, tsl])
            nc.vector.stream_shuffle(qsw[:], qs[:], SHUF_MASK)
            nc.gpsimd.tensor_add(out=dst[:], in0=dst[:], in1=qsw[:])

        def v_mms(ps, jt, ko_range):
            for ko in ko_range:
                nc.tensor.matmul(
                    ps[:, :384], lhsT=xT[:, ko, jt * P : (jt + 1) * P],
                    rhs=wv[:, ko, :], start=(ko == 0), stop=(ko == 5))

        def v_evac(ps, jt, engine):
            out = Vt[:, jt, :, : D]
            src = ps[:, :384].rearrange("p (h d) -> p h d", h=HPC)
            if engine == "act":
                nc.scalar.copy(out=out, in_=src)
            else:
                nc.vector.tensor_copy(out=out, in_=src)

        # ==== phase A (minimal): pair-0 k all chunks + q first half ====
        # first scores only need kT pair0 (k-tiles stream jt=0..15) and
        # qT pair0 q<1024; everything else is deadline-scheduled into
        # attention PE slack
        for tcu in range(NTC):
            tsl = slice(tcu * TC, (tcu + 1) * TC)
            mts = (3, 0) if tcu < 2 else (3,)
            for mt in mts:
                dst = qT if mt < 3 else kT
                hp = mt if mt < 3 else mt - 3
                ps = st_p.tile([P, QH], f32, tag="st", name="psA")
                qk_mms(ps[:, :TC], mt, tsl)
                rope_chain(ps[:, :TC], mt, tsl, dst[:, hp, tsl], "act")

        # =================== deferred-work pieces ===================
        # single deadline-ordered queue; each step pops pieces by earliest
        # deadline while a per-step PE-cost budget remains
        import heapq
        sched = []
        seqn = [0]

        def enq(deadline, cost, fn):
            heapq.heappush(sched, (deadline, seqn[0], cost, fn))
            seqn[0] += 1

        dp_ctr = [0]

        def defer_ps():
            dp_ctr[0] += 1
            if dp_ctr[0] % 2:
                return pj_p.tile([P, TC], f32, tag="pj", name="dps")
            return aux_p.tile([P, TC], f32, tag="trp", name="dps")

        def defer_v(jt, deadline):
            state = {}
            def part1():
                ps = defer_ps()
                state["ps"] = ps
                for ko in range(3):
                    nc.tensor.matmul(
                        ps[:, :384], lhsT=xT[:, ko, jt * P : (jt + 1) * P],
                        rhs=wv[:, ko, :], start=(ko == 0), stop=False)
            def part2():
                ps = state["ps"]
                for ko in range(3, 6):
                    nc.tensor.matmul(
                        ps[:, :384], lhsT=xT[:, ko, jt * P : (jt + 1) * P],
                        rhs=wv[:, ko, :], start=False, stop=(ko == 5))
                v_evac(ps, jt, "dve")
            enq(deadline, 480, part1)
            enq(deadline, 480, part2)

        def defer_qk(mt, tcu, deadline):
            tsl = slice(tcu * TC, (tcu + 1) * TC)
            dst = qT if mt < 3 else kT
            hp = mt if mt < 3 else mt - 3
            state = {}
            def part1():
                ps = defer_ps()
                state["ps"] = ps
                for ko in range(3):
                    nc.tensor.matmul(
                        ps, lhsT=wqk[:, ko, mt * P : (mt + 1) * P],
                        rhs=xT[:, ko, tsl], start=(ko == 0), stop=False)
            def part2():
                ps = state["ps"]
                for ko in range(3, 6):
                    nc.tensor.matmul(
                        ps, lhsT=wqk[:, ko, mt * P : (mt + 1) * P],
                        rhs=xT[:, ko, tsl], start=False, stop=(ko == 5))
                rope_chain(ps, mt, tsl, dst[:, hp, tsl], "dve")
            enq(deadline, 640, part1)
            enq(deadline, 640, part2)

        for tcu in (2, 3):              # pair0 q second half (pass-2 windows)
            defer_qk(0, tcu, 20)
        for jt in range(NT):            # V tiles: strictly before window-0's
            defer_v(jt, 15 + jt // 2)   # AV groups (deadline 24)
        for mt in (4, 1):               # pair 1 (windows 4,5 = step 64)
            for tcu in range(NTC):
                defer_qk(mt, tcu, 52)
        for mt in (5, 2):               # pair 2 (windows 8,9 = step 128)
            for tcu in range(NTC):
                defer_qk(mt, tcu, 112)

        def ya_piece(tt, ch):
            # pair-0 + pair-1 projection contribution for second-half token
            # tiles; pair-2 lands in the tail, host sums y + y2
            def piece():
                ps = defer_ps()
                for ko in range(2):
                    nc.tensor.matmul(
                        ps[:, :384],
                        lhsT=attnT[:, ko, tt * P : (tt + 1) * P],
                        rhs=wp[:, ko, ch * 384 : (ch + 1) * 384],
                        start=(ko == 0), stop=(ko == 1))
                yt = yt_p.tile([P, 384], bf16, tag="yt")
                nc.vector.tensor_copy(out=yt[:], in_=ps[:, :384])
                nc.sync.dma_start(
                    out=y2_d[(tt - NSUB) * P : (tt - NSUB + 1) * P,
                             ch * 384 : (ch + 1) * 384],
                    in_=yt[:])
            return piece

        for tt in range(NSUB, NT):
            for ch in range(2):
                enq(144, 320, ya_piece(tt, ch))

        def proj_piece(tt, ch, pool, stage_engine):
            def piece():
                if pool is pj_p:
                    ps = pj_p.tile([P, TC], f32, tag="pj", name="pjps")
                else:
                    ps = st_p.tile([P, QH], f32, tag="st", name="pjst")
                for ko in range(3):
                    nc.tensor.matmul(
                        ps[:, :384],
                        lhsT=attnT[:, ko, tt * P : (tt + 1) * P],
                        rhs=wp[:, ko, ch * 384 : (ch + 1) * 384],
                        start=(ko == 0), stop=(ko == 2))
                yt = yt_p.tile([P, 384], bf16, tag="yt")
                if stage_engine == "act":
                    nc.scalar.copy(out=yt[:], in_=ps[:, :384])
                else:
                    nc.vector.tensor_copy(out=yt[:], in_=ps[:, :384])
                nc.sync.dma_start(
                    out=y_d[tt * P : (tt + 1) * P, ch * 384 : (ch + 1) * 384],
                    in_=yt[:])
            return piece

        # ============== attention: two q-half passes ==============
        # PSUM accumulation groups zero a whole 2KB bank on start, so the 8
        # AV accumulations per (head, pass) run SEQUENTIALLY (subtile-outer,
        # jt-inner) over the two acc banks. The jt loop emits scores+exp only
        # (keeping all 16 pt tiles); AV groups + normalize/transpose chains
        # are drip-fed into the next head's steps.
        def av_group(h, i, accs, pts, three_way=False):
            def piece():
                if three_way and i % 3 == 2:
                    acc = pj_p.tile([P, D + 1], f32, tag="pj", name="acc")
                elif three_way:
                    acc = acc_p.tile([P, D + 1], f32, tag=f"a{i % 3}", name="acc")
                else:
                    acc = acc_p.tile([P, D + 1], f32, tag=f"a{i % 2}", name="acc")
                accs[i] = acc
                for jt in range(NT):
                    nc.tensor.matmul(
                        acc[:], lhsT=pts[jt][:, i * P : (i + 1) * P],
                        rhs=Vt[:, jt, h, :],
                        start=(jt == 0), stop=(jt == NT - 1))
            return piece

        def chain_piece(h, psx, i, accs):
            hp, base = h // 2, 64 * (h % 2)
            tt = psx * NSUB + i
            def piece():
                acc = accs[i]
                rec = rec_p.tile([P, 1], f32, tag="rec", name="rec")
                nc.vector.reciprocal(out=rec[:], in_=acc[:, D : D + 1])
                anorm = an_p.tile([P, D], bf16, tag="an", name="anorm")
                nc.vector.tensor_scalar_mul(
                    out=anorm[:], in0=acc[:, :D], scalar1=rec[:])
                trp = aux_p.tile([P, P], bf16, tag="trp", name="trp")
                nc.tensor.transpose(
                    trp[base : base + D, :], anorm[:], ident[:],
                    tile_position=(0, base))
                nc.vector.tensor_copy(
                    out=attnT[base : base + D, hp, tt * P : (tt + 1) * P],
                    in_=trp[base : base + D, :])
            return piece

        # window order interleaves the two q-half passes per head pair so
        # pair-1 qk is not needed until step 64 and pair-2 until step 128
        WINDOWS = [(0, 0), (0, 1), (1, 0), (1, 1), (0, 2), (0, 3),
                   (1, 2), (1, 3), (0, 4), (0, 5), (1, 4), (1, 5)]
        for w, (psx, h) in enumerate(WINDOWS):
            hp, base = h // 2, 64 * (h % 2)
            pts = []
            for jt in range(NT):
                st = st_p.tile([P, QH], f32, tag="st", name="st")
                for qc in range(2):
                    q0 = psx * QH + qc * TC
                    nc.tensor.matmul(
                        st[:, qc * TC : (qc + 1) * TC],
                        lhsT=kT[base : base + D, hp, jt * P : (jt + 1) * P],
                        rhs=qT[base : base + D, hp, q0 : q0 + TC],
                        start=True, stop=True,
                        tile_position=(base, 0))
                pt = pt_p.tile([P, QH], bf16, tag="pt")
                nc.scalar.activation(pt[:], st[:], EXP, scale=0.125)
                pts.append(pt)
                spent = 0
                while sched and spent < 450:
                    _, _, c, fn = heapq.heappop(sched)
                    fn()
                    spent += c
            accs = {}
            if w == len(WINDOWS) - 1:
                # drain remaining pieces BEFORE claiming the acc banks for
                # the tail-head groups (rotation must match program order)
                while sched:
                    heapq.heappop(sched)[3]()
                av_group(h, 0, accs, pts, three_way=True)()
                av_group(h, 1, accs, pts, three_way=True)()
                tail_work = (h, psx, accs, pts)
            else:
                dl = (w + 1) * NT + 12
                for i in range(NSUB):
                    enq(dl, 432, av_group(h, i, accs, pts))
                    enq(dl, 100, chain_piece(h, psx, i, accs))
            if w == 9:
                # pass-1 attnT (tts 0..7) complete once window 9's chains
                # drain; project them during the last two windows
                for tt in range(NSUB):
                    for ch in range(2):
                        enq(176, 480, proj_piece(tt, ch, pj_p, "dve"))

        # ---- tail: h5/pass2 AV groups 2..7 + chain + 2nd-half projection;
        # groups 0,1 already ran inline, so chain/proj for them start at once
        # while group i+2 accumulates in the bank freed by chain i ----
        h, psx, accs, pts = tail_work
        av_group(h, 2, accs, pts, three_way=True)()
        chain_piece(h, psx, 0, accs)()
        for i in range(NSUB):
            if i + 1 < NSUB:
                chain_piece(h, psx, i + 1, accs)()
            if i + 3 < NSUB:
                av_group(h, i + 3, accs, pts, three_way=True)()
            tt = NSUB + i
            pj = st_p.tile([P, QH], f32, tag="st", name="pjt")
            for ch in range(2):
                nc.tensor.matmul(
                    pj[:, ch * TC : ch * TC + 384],
                    lhsT=attnT[:, 2, tt * P : (tt + 1) * P],
                    rhs=wp[:, 2, ch * 384 : (ch + 1) * 384],
                    start=True, stop=True)
            yt = yt_p.tile([P, 768], bf16, tag="yt2", name="yt2")
            src_ap = pj[:, : 2 * TC].rearrange("p (b x) -> p b x", b=2)[:, :, :384]
            eng = nc.scalar if i % 2 else nc.vector
            if i % 2:
                nc.scalar.copy(out=yt.rearrange("p (b x) -> p b x", b=2), in_=src_ap)
            else:
                nc.vector.tensor_copy(out=yt.rearrange("p (b x) -> p b x", b=2),
                                      in_=src_ap)
            dq = nc.scalar if i % 2 == 0 else nc.sync
            dq.dma_start(out=y_d[tt * P : (tt + 1) * P, :], in_=yt[:])
        while sched:
            heapq.heappop(sched)[3]()

        if debug:
            nc.sync.dma_start(dbg_q[:], qT[:])
            nc.sync.dma_start(dbg_k[:], kT[:])
            nc.sync.dma_start(dbg_v[:], Vt[:])
            nc.sync.dma_start(dbg_a[:], attnT[:])

    nc.finalize()
    return nc


def _host_inputs(x, rope_cos, rope_sin, W_qkv, b_qkv, W_proj, b_proj, num_special):
    ns = int(num_special)
    bf = ml_dtypes.bfloat16
    cos_pad = np.ones((N, D), np.float32)
    sin_pad = np.zeros((N, D), np.float32)
    cos_pad[ns:] = rope_cos
    sin_pad[ns:] = rope_sin
    m2s = np.empty_like(sin_pad)
    m2s[:, : D // 2] = sin_pad[:, D // 2 :]
    m2s[:, D // 2 :] = -sin_pad[:, : D // 2]
    cos_tab = np.tile(np.ascontiguousarray(cos_pad.T[P64]), (2, 1)).astype(bf)
    m2s_tab = np.tile(np.ascontiguousarray(m2s.T[P64]), (2, 1)).astype(bf)
    ident = np.eye(P, dtype=np.float32).astype(bf)

    in_maps = []
    for core in range(8):
        b, g = core // 2, core % 2
        hs = list(range(HPC * g, HPC * g + HPC))
        cols_qk = []
        for mt in range(6):
            s, hp = (0, mt) if mt < 3 else (1, mt - 3)
            for half in range(2):
                h = hs[2 * hp + half]
                cols_qk.extend(s * 768 + h * 64 + int(P64[p]) for p in range(D))
        cols_qk = np.array(cols_qk)
        cols_v = np.array([2 * 768 + hs[i // 64] * 64 + (i % 64) for i in range(384)])
        rows_p = np.array(
            [hs[2 * ko + half] * 64 + d
             for ko in range(3) for half in range(2) for d in range(D)]
        )
        in_maps.append({
            "xT": np.ascontiguousarray(x[b].T).astype(bf),
            "w_qk": np.ascontiguousarray(
                W_qkv[:, cols_qk].reshape(6, P, 768).transpose(1, 0, 2)).astype(bf),
            "w_v": np.ascontiguousarray(
                W_qkv[:, cols_v].reshape(6, P, 384).transpose(1, 0, 2)).astype(bf),
            "w_p": np.ascontiguousarray(
                W_proj[rows_p].reshape(3, P, 768).transpose(1, 0, 2)).astype(bf),
            "b_qk_t": np.ascontiguousarray(b_qkv[cols_qk].reshape(6, P).T),
            "cos_tab": cos_tab,
            "m2s_tab": m2s_tab,
            "ident": ident,
        })
    return in_maps


def kernel(x, rope_cos, rope_sin, W_qkv, b_qkv, W_proj, b_proj, num_special):
    global LAST_RESULTS
    from concourse.bass_utils import run_bass_kernel_spmd

    x = np.asarray(x, np.float32)
    W_qkv = np.asarray(W_qkv, np.float32)
    b_qkv = np.asarray(b_qkv, np.float32)
    W_proj = np.asarray(W_proj, np.float32)
    b_proj = np.asarray(b_proj, np.float32)
    if "nc" not in _NC_CACHE:
        _NC_CACHE["nc"] = _build_nc()
    nc = _NC_CACHE["nc"]

    in_maps = _host_inputs(
        x, np.asarray(rope_cos, np.float32), np.asarray(rope_sin, np.float32),
        W_qkv, b_qkv, W_proj, b_proj, num_special,
    )
    trace = bool(int(os.environ.get("KERNEL_TRACE", "0")))
    res = run_bass_kernel_spmd(nc, in_maps, core_ids=list(range(8)), trace=trace)
    LAST_RESULTS = res

    bias = b_proj + b_qkv[2 * 768 :] @ W_proj
    out = np.empty((B, N, C), np.float32)
    for b in range(B):
        r0, r1 = res.results[2 * b], res.results[2 * b + 1]
        out[b] = (r0["y"].astype(np.float32)
                  + r1["y"].astype(np.float32) + bias)
        out[b, N // 2 :] += (r0["y2"].astype(np.float32)
                             + r1["y2"].astype(np.float32))
    return out


# revision 70
# speedup vs baseline: 1.2410x; 1.0016x over previous
"""Trainium2 Bass kernel for nn_Attention_49134425866421 (v4, 246560ns).

Dense transformer attention block:
  qkv = x @ W_qkv + b_qkv -> partial RoPE on q,k -> softmax attention -> out proj.

Shapes (hardcoded): B=4, N=2048, C=768, H=12, D=64, fp32 in/out.

Sharding: 8 cores = (batch b in 0..3) x (head-group g in 0..1, 6 heads each).
Host sums the two partials per batch and adds b_proj + b_v @ W_proj (softmax
rows sum to 1, so the device-side V path drops its bias; its projection
contribution is the host-side constant b_v @ W_proj).

Design (vs 304us v1 baseline; ACT exp is the critical engine at 199us):
 - all-bf16 datapath; qk bias folded into the PSUM evacuation via an
   ACT Identity-with-bias (or DVE tensor_scalar_add) op.
 - RoPE rotate_half via DVE stream_shuffle: head-dim layout permuted
   host-side (P64) so rotate pairs sit 16 partitions apart within one
   32-partition block; sign folded into the m2s table.
 - attention as 12 windows interleaving the two q-half passes per head pair
   ([p0 ps1 h0,h1, p0 ps2 h0,h1, p1 ...]) so pair-1 qk projection is not
   needed until step 64 and pair-2 until step 128. Per (window, k-tile):
   2 score matmuls -> [128,1024] PSUM (2 banks, double-buffered) -> one
   1024-wide exp -> pt (bf16). AV runs TRANSPOSED (out[q128, d+rowsum],
   free dim 65) as 8 sequential per-subtile accumulation groups over the
   two acc banks (PSUM start=True zeroes a whole 2KB bank, so groups must
   never share a bank while open); rowsum rides V's ones column.
 - per-subtile chain: reciprocal + per-partition normalize, PE transpose
   (identity matmul, tile_position col offset for odd heads) back to
   head-major attnT.
 - all remaining work (V tiles, qk pairs 0b/1/2, AV groups, chains, early
   projections) is drip-fed into attention PE slack by a deadline-ordered
   scheduler with a per-step PE-cost budget (450ns). Projection is split:
   token tiles 0..7 fully + tiles 8..15 pair-0/1 partials (to y2) run
   during late windows; the tail only computes the pair-2 contribution.
PSUM: st 2x2 banks + acc 2 + aux(trp/deferred) 1 + pj(deferred/acc3) 1 = 8.
"""

import os
import sys

import numpy as np

try:
    import concourse.bass as bass  # noqa: F401
except ImportError:
    sys.path.insert(0, "/opt/trn_rl_repo")

import ml_dtypes

B, N, C, H, D = 4, 2048, 768, 12, 64
HPC = 6          # heads per core
NPAIR = 3
P = 128
NT = N // P      # 16 token tiles
TC = 512
NTC = N // TC    # 4
QH = 1024        # q-half width
NSUB = 8         # q-subtiles per pass

P64 = np.concatenate([np.arange(0, 16), np.arange(32, 48),
                      np.arange(16, 32), np.arange(48, 64)])
SHUF_MASK = [(i + 16) % 32 for i in range(32)]

_NC_CACHE = {}
LAST_RESULTS = None


def _build_nc():
    from contextlib import ExitStack

    import concourse.bass as bass
    import concourse.bacc as bacc
    import concourse.mybir as mybir
    import concourse.tile as tile

    f32 = mybir.dt.float32
    bf16 = mybir.dt.bfloat16
    EXP = mybir.ActivationFunctionType.Exp
    IDENT = mybir.ActivationFunctionType.Identity

    nc = bacc.Bacc(None, target_bir_lowering=False)

    xT_d = nc.dram_tensor("xT", [C, N], bf16, kind="ExternalInput")
    wqk_d = nc.dram_tensor("w_qk", [P, 6, 768], bf16, kind="ExternalInput")
    wv_d = nc.dram_tensor("w_v", [P, 6, 384], bf16, kind="ExternalInput")
    wp_d = nc.dram_tensor("w_p", [P, 3, 768], bf16, kind="ExternalInput")
    bqkt_d = nc.dram_tensor("b_qk_t", [P, 6], f32, kind="ExternalInput")
    cos_d = nc.dram_tensor("cos_tab", [P, N], bf16, kind="ExternalInput")
    m2s_d = nc.dram_tensor("m2s_tab", [P, N], bf16, kind="ExternalInput")
    ident_d = nc.dram_tensor("ident", [P, P], bf16, kind="ExternalInput")
    y_d = nc.dram_tensor("y", [N, C], bf16, kind="ExternalOutput")
    y2_d = nc.dram_tensor("y2", [N // 2, C], bf16, kind="ExternalOutput")
    debug = bool(int(os.environ.get("KERNEL_DEBUG", "0")))
    if debug:
        dbg_q = nc.dram_tensor("dbg_q", [P, NPAIR, N], bf16, kind="ExternalOutput")
        dbg_k = nc.dram_tensor("dbg_k", [P, NPAIR, N], bf16, kind="ExternalOutput")
        dbg_v = nc.dram_tensor("dbg_v", [P, NT, HPC, D + 1], bf16,
                               kind="ExternalOutput")
        dbg_a = nc.dram_tensor("dbg_a", [P, NPAIR, N], bf16, kind="ExternalOutput")

    xT_r = xT_d.rearrange("(ko p) t -> p ko t", p=P)

    with tile.TileContext(nc) as tc, ExitStack() as ctx:
        singles = ctx.enter_context(tc.tile_pool(name="singles", bufs=1))
        rope_p = ctx.enter_context(tc.tile_pool(name="rope", bufs=2))
        pt_p = ctx.enter_context(tc.tile_pool(name="ptp", bufs=36))
        an_p = ctx.enter_context(tc.tile_pool(name="anp", bufs=10))
        yt_p = ctx.enter_context(tc.tile_pool(name="ytp", bufs=4))
        rec_p = ctx.enter_context(tc.tile_pool(name="recp", bufs=2))
        st_p = ctx.enter_context(tc.tile_pool(name="stp", bufs=2, space="PSUM"))
        acc_p = ctx.enter_context(tc.tile_pool(name="accp", bufs=1, space="PSUM"))
        aux_p = ctx.enter_context(tc.tile_pool(name="auxp", bufs=1, space="PSUM"))
        pj_p = ctx.enter_context(tc.tile_pool(name="pjp", bufs=1, space="PSUM"))

        xT = singles.tile([P, 6, N], bf16)
        wqk = singles.tile([P, 6, 768], bf16)
        wv = singles.tile([P, 6, 384], bf16)
        wp = singles.tile([P, 3, 768], bf16)
        bqkt = singles.tile([P, 6], f32)
        cosT = singles.tile([P, N], bf16)
        m2sT = singles.tile([P, N], bf16)
        ident = singles.tile([P, P], bf16)
        qT = singles.tile([P, NPAIR, N], bf16)
        kT = singles.tile([P, NPAIR, N], bf16)
        Vt = singles.tile([P, NT, HPC, D + 1], bf16)
        attnT = singles.tile([P, NPAIR, N], bf16)

        # ---- input DMAs: wqk on ACT queue; xT+tables chunk-interleaved on SP
        # so device-FIFO arrival matches phase-A consumption order ----
        nc.scalar.dma_start(bqkt[:], bqkt_d[:])
        for ko in range(6):
            nc.scalar.dma_start(wqk[:, ko, :], wqk_d[:, ko, :])
        for tcu in range(NTC):
            tsl = slice(tcu * TC, (tcu + 1) * TC)
            for ko in range(6):
                nc.sync.dma_start(xT[:, ko, tsl], xT_r[:, ko, tsl])
            nc.sync.dma_start(cosT[:, tsl], cos_d[:, tsl])
            nc.sync.dma_start(m2sT[:, tsl], m2s_d[:, tsl])
        nc.sync.dma_start(wv[:], wv_d[:])
        nc.sync.dma_start(ident[:], ident_d[:])
        nc.sync.dma_start(wp[:], wp_d[:])

        nc.gpsimd.memset(Vt[:], 1.0)

        # p-state warmup: keep the PE busy from ~0.5us so it reaches full
        # clock (3us ramp) before the real matmuls arrive
        wps = pj_p.tile([P, TC], f32, tag="pj", name="wps")
        for _ in range(40):
            nc.tensor.matmul(wps[:6, :6], lhsT=bqkt[:1, :6], rhs=bqkt[:1, :6],
                             start=True, stop=True)

        def qk_mms(ps, mt, tsl):
            for ko in range(6):
                nc.tensor.matmul(
                    ps, lhsT=wqk[:, ko, mt * P : (mt + 1) * P],
                    rhs=xT[:, ko, tsl], start=(ko == 0), stop=(ko == 5))

        def rope_chain(ps, mt, tsl, dst, pb_engine):
            # PSUM evacuation + per-partition qk bias in one op
            pb = rope_p.tile([P, TC], bf16, tag="pb", bufs=12)
            if pb_engine == "act":
                nc.scalar.activation(pb[:], ps[:], IDENT,
                                     bias=bqkt[:, mt : mt + 1])
            else:
                nc.vector.tensor_scalar_add(out=pb[:], in0=ps[:],
                                            scalar1=bqkt[:, mt : mt + 1])
            qs = rope_p.tile([P, TC], bf16, tag="qs")
            qsw = rope_p.tile([P, TC], bf16, tag="qsw")
            nc.vector.tensor_mul(out=qs[:], in0=pb[:], in1=m2sT[:, tsl])
            nc.vector.tensor_mul(out=dst[:], in0=pb[:], in1=cosT[:, tsl])
            nc.vector.stream_shuffle(qsw[:], qs[:], SHUF_MASK)
            nc.gpsimd.tensor_add(out=dst[:], in0=dst[:], in1=qsw[:])

        def v_mms(ps, jt, ko_range):
            for ko in ko_range:
                nc.tensor.matmul(
                    ps[:, :384], lhsT=xT[:, ko, jt * P : (jt + 1) * P],
                    rhs=wv[:, ko, :], start=(ko == 0), stop=(ko == 5))

        def v_evac(ps, jt, engine):
            out = Vt[:, jt, :, : D]
            src = ps[:, :384].rearrange("p (h d) -> p h d", h=HPC)
            if engine == "act":
                nc.scalar.copy(out=out, in_=src)
            else:
                nc.vector.tensor_copy(out=out, in_=src)

        # ==== phase A (minimal): pair-0 k all chunks + q first half ====
        # first scores only need kT pair0 (k-tiles stream jt=0..15) and
        # qT pair0 q<1024; everything else is deadline-scheduled into
        # attention PE slack
        for tcu in range(NTC):
            tsl = slice(tcu * TC, (tcu + 1) * TC)
            mts = (3, 0) if tcu < 2 else (3,)
            for mt in mts:
                dst = qT if mt < 3 else kT
                hp = mt if mt < 3 else mt - 3
                ps = st_p.tile([P, QH], f32, tag="st", name="psA")
                qk_mms(ps[:, :TC], mt, tsl)
                rope_chain(ps[:, :TC], mt, tsl, dst[:, hp, tsl], "act")

        # =================== deferred-work pieces ===================
        # single deadline-ordered queue; each step pops pieces by earliest
        # deadline while a per-step PE-cost budget remains
        import heapq
        sched = []
        seqn = [0]

        def enq(deadline, cost, fn):
            heapq.heappush(sched, (deadline, seqn[0], cost, fn))
            seqn[0] += 1

        dp_ctr = [0]

        def defer_ps():
            dp_ctr[0] += 1
            if dp_ctr[0] % 2:
                return pj_p.tile([P, TC], f32, tag="pj", name="dps")
            return aux_p.tile([P, TC], f32, tag="trp", name="dps")

        def defer_v(jt, deadline):
            state = {}
            def part1():
                ps = defer_ps()
                state["ps"] = ps
                for ko in range(3):
                    nc.tensor.matmul(
                        ps[:, :384], lhsT=xT[:, ko, jt * P : (jt + 1) * P],
                        rhs=wv[:, ko, :], start=(ko == 0), stop=False)
            def part2():
                ps = state["ps"]
                for ko in range(3, 6):
                    nc.tensor.matmul(
                        ps[:, :384], lhsT=xT[:, ko, jt * P : (jt + 1) * P],
                        rhs=wv[:, ko, :], start=False, stop=(ko == 5))
                v_evac(ps, jt, "dve")
            enq(deadline, 480, part1)
            enq(deadline, 480, part2)

        def defer_qk(mt, tcu, deadline):
            tsl = slice(tcu * TC, (tcu + 1) * TC)
            dst = qT if mt < 3 else kT
            hp = mt if mt < 3 else mt - 3
            state = {}
            def part1():
                ps = defer_ps()
                state["ps"] = ps
                for ko in range(3):
                    nc.tensor.matmul(
                        ps, lhsT=wqk[:, ko, mt * P : (mt + 1) * P],
                        rhs=xT[:, ko, tsl], start=(ko == 0), stop=False)
            def part2():
                ps = state["ps"]
                for ko in range(3, 6):
                    nc.tensor.matmul(
                        ps, lhsT=wqk[:, ko, mt * P : (mt + 1) * P],
                        rhs=xT[:, ko, tsl], start=False, stop=(ko == 5))
                rope_chain(ps, mt, tsl, dst[:, hp, tsl], "dve")
            enq(deadline, 640, part1)
            enq(deadline, 640, part2)

        for tcu in (2, 3):              # pair0 q second half (pass-2 windows)
            defer_qk(0, tcu, 20)
        for jt in range(NT):            # V tiles: strictly before window-0's
            defer_v(jt, 15 + jt // 2)   # AV groups (deadline 24)
        for mt in (4, 1):               # pair 1 (windows 4,5 = step 64)
            for tcu in range(NTC):
                defer_qk(mt, tcu, 52)
        for mt in (5, 2):               # pair 2 (windows 8,9 = step 128)
            for tcu in range(NTC):
                defer_qk(mt, tcu, 112)

        def ya_piece(tt, ch):
            # pair-0 + pair-1 projection contribution for second-half token
            # tiles; pair-2 lands in the tail, host sums y + y2
            def piece():
                ps = defer_ps()
                for ko in range(2):
                    nc.tensor.matmul(
                        ps[:, :384],
                        lhsT=attnT[:, ko, tt * P : (tt + 1) * P],
                        rhs=wp[:, ko, ch * 384 : (ch + 1) * 384],
                        start=(ko == 0), stop=(ko == 1))
                yt = yt_p.tile([P, 384], bf16, tag="yt")
                nc.vector.tensor_copy(out=yt[:], in_=ps[:, :384])
                nc.sync.dma_start(
                    out=y2_d[(tt - NSUB) * P : (tt - NSUB + 1) * P,
                             ch * 384 : (ch + 1) * 384],
                    in_=yt[:])
            return piece

        for tt in range(NSUB, NT):
            for ch in range(2):
                enq(144, 320, ya_piece(tt, ch))

        def proj_piece(tt, ch, pool, stage_engine):
            def piece():
                if pool is pj_p:
                    ps = pj_p.tile([P, TC], f32, tag="pj", name="pjps")
                else:
                    ps = st_p.tile([P, QH], f32, tag="st", name="pjst")
                for ko in range(3):
                    nc.tensor.matmul(
                        ps[:, :384],
                        lhsT=attnT[:, ko, tt * P : (tt + 1) * P],
                        rhs=wp[:, ko, ch * 384 : (ch + 1) * 384],
                        start=(ko == 0), stop=(ko == 2))
                yt = yt_p.tile([P, 384], bf16, tag="yt")
                if stage_engine == "act":
                    nc.scalar.copy(out=yt[:], in_=ps[:, :384])
                else:
                    nc.vector.tensor_copy(out=yt[:], in_=ps[:, :384])
                nc.sync.dma_start(
                    out=y_d[tt * P : (tt + 1) * P, ch * 384 : (ch + 1) * 384],
                    in_=yt[:])
            return piece

        # ============== attention: two q-half passes ==============
        # PSUM accumulation groups zero a whole 2KB bank on start, so the 8
        # AV accumulations per (head, pass) run SEQUENTIALLY (subtile-outer,
        # jt-inner) over the two acc banks. The jt loop emits scores+exp only
        # (keeping all 16 pt tiles); AV groups + normalize/transpose chains
        # are drip-fed into the next head's steps.
        def av_group(h, i, accs, pts, three_way=False):
            def piece():
                if three_way and i % 3 == 2:
                    acc = pj_p.tile([P, D + 1], f32, tag="pj", name="acc")
                elif three_way:
                    acc = acc_p.tile([P, D + 1], f32, tag=f"a{i % 3}", name="acc")
                else:
                    acc = acc_p.tile([P, D + 1], f32, tag=f"a{i % 2}", name="acc")
                accs[i] = acc
                for jt in range(NT):
                    nc.tensor.matmul(
                        acc[:], lhsT=pts[jt][:, i * P : (i + 1) * P],
                        rhs=Vt[:, jt, h, :],
                        start=(jt == 0), stop=(jt == NT - 1))
            return piece

        def chain_piece(h, psx, i, accs):
            hp, base = h // 2, 64 * (h % 2)
            tt = psx * NSUB + i
            def piece():
                acc = accs[i]
                rec = rec_p.tile([P, 1], f32, tag="rec", name="rec")
                nc.vector.reciprocal(out=rec[:], in_=acc[:, D : D + 1])
                anorm = an_p.tile([P, D], bf16, tag="an", name="anorm")
                nc.vector.tensor_scalar_mul(
                    out=anorm[:], in0=acc[:, :D], scalar1=rec[:])
                trp = aux_p.tile([P, P], bf16, tag="trp", name="trp")
                nc.tensor.transpose(
                    trp[base : base + D, :], anorm[:], ident[:],
                    tile_position=(0, base))
                nc.vector.tensor_copy(
                    out=attnT[base : base + D, hp, tt * P : (tt + 1) * P],
                    in_=trp[base : base + D, :])
            return piece

        # window order interleaves the two q-half passes per head pair so
        # pair-1 qk is not needed until step 64 and pair-2 until step 128
        WINDOWS = [(0, 0), (0, 1), (1, 0), (1, 1), (0, 2), (0, 3),
                   (1, 2), (1, 3), (0, 4), (0, 5), (1, 4), (1, 5)]
        for w, (psx, h) in enumerate(WINDOWS):
            hp, base = h // 2, 64 * (h % 2)
            pts = []
            for jt in range(NT):
                st = st_p.tile([P, QH], f32, tag="st", name="st")
                for qc in range(2):
                    q0 = psx * QH + qc * TC
                    nc.tensor.matmul(
                        st[:, qc * TC : (qc + 1) * TC],
                        lhsT=kT[base : base + D, hp, jt * P : (jt + 1) * P],
                        rhs=qT[base : base + D, hp, q0 : q0 + TC],
                        start=True, stop=True,
                        tile_position=(base, 0))
                pt = pt_p.tile([P, QH], bf16, tag="pt")
                nc.scalar.activation(pt[:], st[:], EXP, scale=0.125)
                pts.append(pt)
                spent = 0
                while sched and spent < 450:
                    _, _, c, fn = heapq.heappop(sched)
                    fn()
                    spent += c
            accs = {}
            if w == len(WINDOWS) - 1:
                # drain remaining pieces BEFORE claiming the acc banks for
                # the tail-head groups (rotation must match program order)
                while sched:
                    heapq.heappop(sched)[3]()
                av_group(h, 0, accs, pts, three_way=True)()
                av_group(h, 1, accs, pts, three_way=True)()
                tail_work = (h, psx, accs, pts)
            else:
                dl = (w + 1) * NT + 12
                for i in range(NSUB):
                    enq(dl, 432, av_group(h, i, accs, pts))
                    enq(dl, 100, chain_piece(h, psx, i, accs))
            if w == 9:
                # pass-1 attnT (tts 0..7) complete once window 9's chains
                # drain; project them during the last two windows
                for tt in range(NSUB):
                    for ch in range(2):
                        enq(176, 480, proj_piece(tt, ch, pj_p, "dve"))

        # ---- tail: h5/pass2 AV groups 2..7 + chain + 2nd-half projection;
        # groups 0,1 already ran inline, so chain/proj for them start at once
        # while group i+2 accumulates in the bank freed by chain i ----
        h, psx, accs, pts = tail_work
        av_group(h, 2, accs, pts, three_way=True)()
        chain_piece(h, psx, 0, accs)()
        for i in range(NSUB):
            if i + 1 < NSUB:
                chain_piece(h, psx, i + 1, accs)()
            if i + 3 < NSUB:
                av_group(h, i + 3, accs, pts, three_way=True)()
            tt = NSUB + i
            pj = st_p.tile([P, QH], f32, tag="st", name="pjt")
            for ch in range(2):
                nc.tensor.matmul(
                    pj[:, ch * TC : ch * TC + 384],
                    lhsT=attnT[:, 2, tt * P : (tt + 1) * P],
                    rhs=wp[:, 2, ch * 384 : (ch + 1) * 384],
                    start=True, stop=True)
            yt = yt_p.tile([P, 768], bf16, tag="yt2", name="yt2")
            src_ap = pj[:, : 2 * TC].rearrange("p (b x) -> p b x", b=2)[:, :, :384]
            # ACT is idle after the last exp; keep DVE free for the chains
            nc.scalar.copy(out=yt.rearrange("p (b x) -> p b x", b=2), in_=src_ap)
            dq = nc.scalar if i % 2 == 0 else nc.sync
            dq.dma_start(out=y_d[tt * P : (tt + 1) * P, :], in_=yt[:])
        while sched:
            heapq.heappop(sched)[3]()

        if debug:
            nc.sync.dma_start(dbg_q[:], qT[:])
            nc.sync.dma_start(dbg_k[:], kT[:])
            nc.sync.dma_start(dbg_v[:], Vt[:])
            nc.sync.dma_start(dbg_a[:], attnT[:])

    nc.finalize()
    return nc


def _host_inputs(x, rope_cos, rope_sin, W_qkv, b_qkv, W_proj, b_proj, num_special):
    ns = int(num_special)
    bf = ml_dtypes.bfloat16
    cos_pad = np.ones((N, D), np.float32)
    sin_pad = np.zeros((N, D), np.float32)
    cos_pad[ns:] = rope_cos
    sin_pad[ns:] = rope_sin
    m2s = np.empty_like(sin_pad)
    m2s[:, : D // 2] = sin_pad[:, D // 2 :]
    m2s[:, D // 2 :] = -sin_pad[:, : D // 2]
    cos_tab = np.tile(np.ascontiguousarray(cos_pad.T[P64]), (2, 1)).astype(bf)
    m2s_tab = np.tile(np.ascontiguousarray(m2s.T[P64]), (2, 1)).astype(bf)
    ident = np.eye(P, dtype=np.float32).astype(bf)

    in_maps = []
    for core in range(8):
        b, g = core // 2, core % 2
        hs = list(range(HPC * g, HPC * g + HPC))
        cols_qk = []
        for mt in range(6):
            s, hp = (0, mt) if mt < 3 else (1, mt - 3)
            for half in range(2):
                h = hs[2 * hp + half]
                cols_qk.extend(s * 768 + h * 64 + int(P64[p]) for p in range(D))
        cols_qk = np.array(cols_qk)
        cols_v = np.array([2 * 768 + hs[i // 64] * 64 + (i % 64) for i in range(384)])
        rows_p = np.array(
            [hs[2 * ko + half] * 64 + d
             for ko in range(3) for half in range(2) for d in range(D)]
        )
        in_maps.append({
            "xT": np.ascontiguousarray(x[b].T).astype(bf),
            "w_qk": np.ascontiguousarray(
                W_qkv[:, cols_qk].reshape(6, P, 768).transpose(1, 0, 2)).astype(bf),
            "w_v": np.ascontiguousarray(
                W_qkv[:, cols_v].reshape(6, P, 384).transpose(1, 0, 2)).astype(bf),
            "w_p": np.ascontiguousarray(
                W_proj[rows_p].reshape(3, P, 768).transpose(1, 0, 2)).astype(bf),
            "b_qk_t": np.ascontiguousarray(b_qkv[cols_qk].reshape(6, P).T),
            "cos_tab": cos_tab,
            "m2s_tab": m2s_tab,
            "ident": ident,
        })
    return in_maps


def kernel(x, rope_cos, rope_sin, W_qkv, b_qkv, W_proj, b_proj, num_special):
    global LAST_RESULTS
    from concourse.bass_utils import run_bass_kernel_spmd

    x = np.asarray(x, np.float32)
    W_qkv = np.asarray(W_qkv, np.float32)
    b_qkv = np.asarray(b_qkv, np.float32)
    W_proj = np.asarray(W_proj, np.float32)
    b_proj = np.asarray(b_proj, np.float32)
    if "nc" not in _NC_CACHE:
        _NC_CACHE["nc"] = _build_nc()
    nc = _NC_CACHE["nc"]

    in_maps = _host_inputs(
        x, np.asarray(rope_cos, np.float32), np.asarray(rope_sin, np.float32),
        W_qkv, b_qkv, W_proj, b_proj, num_special,
    )
    trace = bool(int(os.environ.get("KERNEL_TRACE", "0")))
    res = run_bass_kernel_spmd(nc, in_maps, core_ids=list(range(8)), trace=trace)
    LAST_RESULTS = res

    bias = b_proj + b_qkv[2 * 768 :] @ W_proj
    out = np.empty((B, N, C), np.float32)
    for b in range(B):
        r0, r1 = res.results[2 * b], res.results[2 * b + 1]
        out[b] = (r0["y"].astype(np.float32)
                  + r1["y"].astype(np.float32) + bias)
        out[b, N // 2 :] += (r0["y2"].astype(np.float32)
                             + r1["y2"].astype(np.float32))
    return out
